# revision 1
# baseline (speedup 1.0000x reference)
"""Trainium2 Bass kernel for nn_CNFAdapter.

Algorithm (mathematically identical to the reference, heavily folded):

  The literal MLP ``h = gelu([ve[v]; se[s]] @ W1.T + b1) @ W2.T + b2`` only
  has 514 distinct inputs (257 vars x 2 signs), so it is folded on the host
  into a table ``T[514, 256]``.  The clause embedding before LayerNorm is
  ``mean_l h = (1/L) * sum_l T[ci_l]``; dividing T by L bakes in the mean,
  and subtracting each table row's d-mean makes the clause vector exactly
  zero-mean, which removes the LN mean term entirely.

  Per instance the device computes (c = clause, d = hidden, hp = (head,query)):
     xT[d, c]   = tableT @ counts       (counts = per-clause literal histogram)
     rs[c]      = 1/sqrt(sum_d x^2 / D + eps)
     s1v[c, :]  = x @ [Wkq | WvF]       (Wkq folds cn_g, Wk, q, softmax scale)
     expT[c,hp] = exp(rs*s1 + maskbias) (unnormalized softmax, max-sub skipped:
                                         scores are O(1e-2); bk dropped via
                                         softmax shift invariance)
     vq[c,he]   = rs * vtmp             (bv folded into the final bias)
     Z[hp]      = sum_c expT
     bigctx     = vq.T @ expT           (diag head-blocks are the context)
     out        = LN(pqb + ctx @ out_w.T) * pn_g + pn_b

  Sharding: data-parallel over B=32 instances, 4 per NeuronCore; all
  parameters replicated (host-folded, ~1 MB).
"""

import math
from contextlib import ExitStack

import numpy as np

import concourse.bass as bass
import concourse.mybir as mybir
import concourse.tile as tile
from concourse import bacc
from concourse.bass_utils import run_bass_kernel_spmd

# ---------------- problem constants (hardcoded) ----------------
D = 256
H = 8
P = 32
V = 257
EPS = 1e-5
B, C, L = 32, 2048, 8
VOC = 2 * V            # 514 combined (var, sign) literals
VCH = 5                # ceil(514/128) contraction chunks (last has K=2)
NCORES = 8
BPC = B // NCORES      # 4 instances per core
CB = C // 128          # 16 chunks of 128 clauses
hd = D // H

fp16 = mybir.dt.float16
fp32 = mybir.dt.float32
AF = mybir.ActivationFunctionType
ALU = mybir.AluOpType
AX = mybir.AxisListType


def _emit(nc, tc, ctx, dr, out_dram):
    pc = ctx.enter_context(tc.tile_pool(name="consts", bufs=1))
    pcnt = ctx.enter_context(tc.tile_pool(name="cnt", bufs=4))
    px = ctx.enter_context(tc.tile_pool(name="x", bufs=2))
    px2 = ctx.enter_context(tc.tile_pool(name="x2", bufs=2))
    pexp = ctx.enter_context(tc.tile_pool(name="expv", bufs=2))
    pst = ctx.enter_context(tc.tile_pool(name="stats", bufs=2))
    psm = ctx.enter_context(tc.tile_pool(name="small", bufs=2))
    ps_mm = ctx.enter_context(tc.tile_pool(name="ps_mm", bufs=3, space="PSUM"))
    ps_st = ctx.enter_context(tc.tile_pool(name="ps_st", bufs=1, space="PSUM"))
    ps_z = ctx.enter_context(tc.tile_pool(name="ps_z", bufs=1, space="PSUM"))
    ps_bc = ctx.enter_context(tc.tile_pool(name="ps_bc", bufs=1, space="PSUM"))
    ps_tail = ctx.enter_context(tc.tile_pool(name="ps_tail", bufs=1, space="PSUM"))

    # ---- constants to SBUF ----
    tbl = pc.tile([128, VCH, D], fp16, tag="tbl")
    nc.sync.dma_start(out=tbl[:], in_=dr["tbls"][:])
    wkv = pc.tile([128, 2, 2 * D], fp16, tag="wkv")
    nc.sync.dma_start(out=wkv[:], in_=dr["wkv"][:])
    owt = pc.tile([128, 2, D], fp32, tag="owt")
    nc.sync.dma_start(out=owt[:], in_=dr["owt"][:])
    pqb = pc.tile([P, D], fp32, tag="pqb")
    nc.sync.dma_start(out=pqb[:], in_=dr["pqb"][:])
    png = pc.tile([P, D], fp32, tag="png")
    nc.sync.dma_start(out=png[:], in_=dr["png"][:])
    pnb = pc.tile([P, D], fp32, tag="pnb")
    nc.sync.dma_start(out=pnb[:], in_=dr["pnb"][:])
    ones16 = pc.tile([128, 1], fp16, tag="ones16")
    nc.sync.dma_start(out=ones16[:], in_=dr["ones16"][:])
    ident = pc.tile([P, P], fp32, tag="ident")
    nc.sync.dma_start(out=ident[:], in_=dr["ident"][:])
    epst = pc.tile([128, 1], fp32, tag="epst")
    nc.vector.memset(epst[:], EPS)

    for b in range(BPC):
        # ---- load per-instance data ----
        cnt = pcnt.tile([128, VCH, C], fp16, tag="cnt")
        nc.sync.dma_start(out=cnt[:], in_=dr["cnt"][b])
        mb_t = psm.tile([128, CB], fp32, tag="mb")
        nc.sync.dma_start(out=mb_t[:], in_=dr["maskb"][b])

        # ---- xT[d%128, d//128, c] = tableT @ counts (fp32 acc -> fp16) ----
        x = px.tile([128, 2, C], fp16, tag="x")
        for cc in range(4):
            csl = slice(cc * 512, (cc + 1) * 512)
            for dh in range(2):
                pxm = ps_mm.tile([128, 512], fp32, tag="mm512")
                for vc in range(VCH):
                    if vc < 4:
                        lhsT = tbl[:, vc, dh * 128:(dh + 1) * 128]
                        rhs = cnt[:, vc, csl]
                    else:
                        lhsT = tbl[0:2, vc, dh * 128:(dh + 1) * 128]
                        rhs = cnt[0:2, vc, csl]
                    nc.tensor.matmul(pxm[:], lhsT=lhsT, rhs=rhs,
                                     start=(vc == 0), stop=(vc == VCH - 1))
                nc.scalar.copy(x[:, dh, csl], pxm[:])

        # ---- stats: rs[c] = 1/sqrt(sum_d x^2 / D + eps) ----
        x2 = px2.tile([128, 2, C], fp16, tag="x2")
        nc.scalar.square(x2[:], x[:])
        pst_t = ps_st.tile([128, CB], fp32, tag="st")
        for cb in range(CB):
            for dh in range(2):
                nc.tensor.matmul(pst_t[:, cb:cb + 1],
                                 lhsT=x2[:, dh, cb * 128:(cb + 1) * 128],
                                 rhs=ones16[:],
                                 start=(dh == 0), stop=(dh == 1))
        ssq = pst.tile([128, CB], fp32, tag="ssq")
        nc.vector.tensor_copy(ssq[:], pst_t[:])
        sq = pst.tile([128, CB], fp32, tag="sq")
        nc.scalar.activation(sq[:], ssq[:], AF.Sqrt, bias=epst[:, 0:1], scale=1.0 / D)
        rs = pst.tile([128, CB], fp32, tag="rs")
        nc.vector.reciprocal(rs[:], sq[:])

        # ---- s1v = x @ [Wkq | WvF]; fused exp / v-scale epilogues ----
        expT = pexp.tile([128, CB, 256], fp16, tag="expT")
        vq = pexp.tile([128, CB, 256], fp16, tag="vq")
        for cb in range(CB):
            sv = ps_mm.tile([128, 512], fp32, tag="mm512")
            for dh in range(2):
                nc.tensor.matmul(sv[:],
                                 lhsT=x[:, dh, cb * 128:(cb + 1) * 128],
                                 rhs=wkv[:, dh, :],
                                 start=(dh == 0), stop=(dh == 1))
            nc.scalar.activation(expT[:, cb, :], sv[:, 0:256], AF.Exp,
                                 bias=mb_t[:, cb:cb + 1], scale=rs[:, cb:cb + 1])
            nc.vector.tensor_scalar_mul(vq[:, cb, :], sv[:, 256:512],
                                        rs[:, cb:cb + 1])

        # ---- Z and bigctx accumulation over all clauses ----
        pz_t = ps_z.tile([1, 256], fp32, tag="z")
        pbc0 = ps_bc.tile([128, 256], fp32, tag="bc0")
        pbc1 = ps_bc.tile([128, 256], fp32, tag="bc1")
        pbc = [pbc0, pbc1]
        for cb in range(CB):
            nc.tensor.matmul(pz_t[:], lhsT=ones16[:], rhs=expT[:, cb, :],
                             start=(cb == 0), stop=(cb == CB - 1))
            for h2 in range(2):
                nc.tensor.matmul(pbc[h2][:],
                                 lhsT=vq[:, cb, h2 * 128:(h2 + 1) * 128],
                                 rhs=expT[:, cb, :],
                                 start=(cb == 0), stop=(cb == CB - 1))

        # ---- 1/Z in [p, h] layout (32x32 block transpose of the Z row) ----
        zpad = psm.tile([P, 256], fp32, tag="zpad")
        nc.vector.tensor_copy(zpad[0:1, :], pz_t[:])
        zptr = psm.tile([P, 256], fp32, tag="zptr")
        nc.vector.transpose(zptr[:], zpad[:])
        zsel = zptr[:].rearrange("p (h q) -> p h q", q=P)
        zp = psm.tile([P, H], fp32, tag="zp")
        nc.vector.reciprocal(zp[:], zsel[:, :, 0])

        # ---- extract diagonal head blocks, scale by 1/Z -> ctx [p, d] ----
        ctx_t = psm.tile([P, D], fp32, tag="ctx")
        for h in range(H):
            h2, hh = divmod(h, 4)
            blk = pbc[h2][hh * 32:(hh + 1) * 32, h * 32:h * 32 + 32]
            tmp = psm.tile([P, P], fp32, tag="ctxblk")
            nc.vector.transpose(tmp[:], blk)
            nc.vector.tensor_scalar_mul(ctx_t[:, h * 32:(h + 1) * 32], tmp[:],
                                        zp[:, h:h + 1])

        # ---- ctxT (PE transpose) ----
        pct_t = ps_tail.tile([128, 2 * P], fp32, tag="tail")
        for dh in range(2):
            nc.tensor.transpose(pct_t[:, dh * P:(dh + 1) * P],
                                ctx_t[:, dh * 128:(dh + 1) * 128], ident[:])
        ctxT = psm.tile([128, 2 * P], fp32, tag="ctxT")
        nc.scalar.copy(ctxT[:], pct_t[:])

        # ---- refined = ctx @ out_w.T + pqb (pqb via identity matmul) ----
        prf_t = ps_tail.tile([P, D], fp32, tag="tail")
        for dh in range(2):
            nc.tensor.matmul(prf_t[:], lhsT=ctxT[:, dh * P:(dh + 1) * P],
                             rhs=owt[:, dh, :], start=(dh == 0), stop=False)
        nc.tensor.matmul(prf_t[:], lhsT=ident[:], rhs=pqb[:],
                         start=False, stop=True)

        # ---- final LayerNorm ----
        ssum = pst.tile([P, 1], fp32, tag="ssum")
        nc.vector.tensor_reduce(ssum[:], prf_t[:], axis=AX.X, op=ALU.add)
        nm = pst.tile([P, 1], fp32, tag="nm")
        nc.vector.tensor_scalar_mul(nm[:], ssum[:], -1.0 / D)
        cen = psm.tile([P, D], fp32, tag="cen")
        nc.scalar.activation(cen[:], prf_t[:], AF.Identity, bias=nm[:, 0:1])
        sq2 = psm.tile([P, D], fp32, tag="sq2")
        nc.vector.tensor_mul(sq2[:], cen[:], cen[:])
        vs = pst.tile([P, 1], fp32, tag="vs")
        nc.vector.tensor_reduce(vs[:], sq2[:], axis=AX.X, op=ALU.add)
        stdv = pst.tile([P, 1], fp32, tag="stdv")
        nc.scalar.activation(stdv[:], vs[:], AF.Sqrt, bias=epst[0:P, 0:1], scale=1.0 / D)
        rstd = pst.tile([P, 1], fp32, tag="rstd")
        nc.vector.reciprocal(rstd[:], stdv[:])
        t1 = psm.tile([P, D], fp32, tag="t1")
        nc.vector.tensor_scalar_mul(t1[:], cen[:], rstd[:, 0:1])
        t2 = psm.tile([P, D], fp32, tag="t2")
        nc.vector.tensor_mul(t2[:], t1[:], png[:])
        outt = psm.tile([P, D], fp32, tag="outt")
        nc.vector.tensor_add(outt[:], t2[:], pnb[:])
        nc.sync.dma_start(out=out_dram[b], in_=outt[:])


def _build_nc():
    nc = bacc.Bacc("TRN2", target_bir_lowering=False, debug=False,
                   num_devices=NCORES)
    dr = {}

    def din(name, shape, dt):
        dr[name] = nc.dram_tensor(name, shape, dt, kind="ExternalInput").ap()

    din("tbls", [128, VCH, D], fp16)
    din("cnt", [BPC, 128, VCH, C], fp16)
    din("wkv", [128, 2, 2 * D], fp16)
    din("owt", [128, 2, D], fp32)
    din("pqb", [P, D], fp32)
    din("png", [P, D], fp32)
    din("pnb", [P, D], fp32)
    din("maskb", [BPC, 128, CB], fp32)
    din("ones16", [128, 1], fp16)
    din("ident", [P, P], fp32)
    out_dram = nc.dram_tensor("out", [BPC, P, D], fp32, kind="ExternalOutput").ap()

    with tile.TileContext(nc) as tc, ExitStack() as ctx:
        _emit(nc, tc, ctx, dr, out_dram)
    nc.compile()
    return nc


_NC_CACHE = None


def _get_nc():
    global _NC_CACHE
    if _NC_CACHE is None:
        _NC_CACHE = _build_nc()
    return _NC_CACHE


def _erf(x):
    try:
        from scipy.special import erf
        return erf(x)
    except Exception:
        from math import erf as _e
        return np.vectorize(_e)(x)


def host_prepare(inputs):
    """Fold weights, build per-core input maps. All in float64 then cast."""
    ve = inputs["var_embed"].astype(np.float64)
    se = inputs["sign_embed"].astype(np.float64)
    W1 = inputs["W1"].astype(np.float64)
    b1 = inputs["b1"].astype(np.float64)
    W2 = inputs["W2"].astype(np.float64)
    b2 = inputs["b2"].astype(np.float64)
    cn_g = inputs["cn_g"].astype(np.float64)
    cn_b = inputs["cn_b"].astype(np.float64)
    pq = inputs["prefix_queries"].astype(np.float64)
    in_w = inputs["in_proj_w"].astype(np.float64)
    in_b = inputs["in_proj_b"].astype(np.float64)
    out_w = inputs["out_w"].astype(np.float64)
    out_b = inputs["out_b"].astype(np.float64)
    pn_g = inputs["pn_g"].astype(np.float64)
    pn_b = inputs["pn_b"].astype(np.float64)

    # literal table over combined index j = v*2 + s; /L bakes the clause mean,
    # row-centering makes clause vectors exactly zero-mean under LN
    lit = np.concatenate([np.repeat(ve, 2, axis=0), np.tile(se, (V, 1))], axis=1)
    z = lit @ W1.T + b1
    gelu = 0.5 * z * (1.0 + _erf(z / math.sqrt(2.0)))
    table = (gelu @ W2.T + b2) / L
    table = table - table.mean(axis=1, keepdims=True)        # [514, D]
    tpad = np.zeros((VCH * 128, D), np.float64)
    tpad[:VOC] = table
    tbls = np.ascontiguousarray(
        tpad.reshape(VCH, 128, D).transpose(1, 0, 2)).astype(np.float16)

    Wq, Wk, Wv = np.split(in_w, 3, axis=0)
    bq, bk, bv = np.split(in_b, 3)
    q = pq @ Wq.T + bq                                       # [P, D]
    scale = 1.0 / math.sqrt(hd)
    WkF = cn_g[:, None] * Wk.T
    WvF = cn_g[:, None] * Wv.T
    qh = q.reshape(P, H, hd)
    qbd = np.zeros((D, H * P))
    for h in range(H):
        qbd[h * hd:(h + 1) * hd, h * P:(h + 1) * P] = qh[:, h, :].T * scale
    WKV = np.concatenate([WkF @ qbd, WvF], axis=1)           # [D, 512]
    wkv = np.ascontiguousarray(
        WKV.reshape(2, 128, 2 * D).transpose(1, 0, 2)).astype(np.float16)

    bvF = cn_b @ Wv.T + bv                                   # bk dropped (softmax shift)
    pqb = (pq + out_b + bvF @ out_w.T).astype(np.float32)
    owt = np.ascontiguousarray(
        out_w.T.reshape(2, 128, D).transpose(1, 0, 2)).astype(np.float32)

    png = np.broadcast_to(pn_g, (P, D)).astype(np.float32)
    pnb = np.broadcast_to(pn_b, (P, D)).astype(np.float32)
    ident = np.eye(P, dtype=np.float32)
    ones16 = np.ones((128, 1), np.float16)

    # per-clause literal histograms, chunk-transposed: cnt[b, v%128, v//128, c]
    ci = (inputs["var_idx"].astype(np.int64) * 2
          + inputs["sign_idx"].astype(np.int64))             # [B, C, L]
    maskb_full = np.where(np.asarray(inputs["mask"]) > 0, 0.0, -1e9)

    in_maps = []
    for core in range(NCORES):
        cnt = np.zeros((BPC, 128, VCH, C), np.float16)
        mkb = np.zeros((BPC, 128, CB), np.float32)
        for bl in range(BPC):
            bg = core * BPC + bl
            flat = ci[bg].reshape(-1)
            rows = np.repeat(np.arange(C, dtype=np.int64), L)
            cc = np.bincount(rows * (VCH * 128) + flat,
                             minlength=C * VCH * 128).reshape(C, VCH * 128)
            cnt[bl] = cc.reshape(C, VCH, 128).transpose(2, 1, 0)
            mkb[bl] = maskb_full[bg].reshape(CB, 128).T
        in_maps.append({
            "tbls": tbls, "cnt": cnt, "wkv": wkv, "owt": owt, "pqb": pqb,
            "png": png, "pnb": pnb, "maskb": mkb, "ones16": ones16,
            "ident": ident,
        })
    return in_maps


def kernel(**inputs):
    nc = _get_nc()
    in_maps = host_prepare(inputs)
    res = run_bass_kernel_spmd(nc, in_maps, core_ids=list(range(NCORES)))
    out = np.concatenate([res.results[i]["out"] for i in range(NCORES)], axis=0)
    return np.ascontiguousarray(out.astype(np.float32))



# revision 6
# speedup vs baseline: 1.1978x; 1.1978x over previous
"""Trainium2 Bass kernel for nn_CNFAdapter (fp8 DoubleRow version).

Algorithm (identical math to the reference, heavily folded):

  The literal MLP ``h = gelu([ve[v]; se[s]] @ W1.T + b1) @ W2.T + b2`` has
  only 514 distinct inputs (257 vars x 2 signs); it is folded on the host
  into a table ``T[514, 256]``.  The clause embedding before LayerNorm is
  ``mean_l h``; dividing T by L bakes in the mean, and subtracting each
  row's d-mean makes clause vectors exactly zero-mean, removing the LN
  mean term.

  Per instance (c = clause, d = hidden, hp = (head,query), he = (head,e)):
     xT[d, c]   = tableT @ counts       (counts = per-clause literal histogram)
     rs[c]      = 1/sqrt(sum_d x^2 / D + eps)
     sv[c, :]   = x @ [Wkq | WvF]       (Wkq folds cn_g, Wk, q, softmax scale)
     expT[c,hp] = exp(rs*sv_kq + maskbias)   (bk dropped: softmax shift inv.)
     vq[c, he]  = rs * sv_v
     bigctx[hp, he+1] = expT.T @ [vq | zconst]   (Z in the extra column)
     ctx[p, he] = diag head blocks / Z
     out        = LN(pqb + ctx @ out_w.T) * pn_g + pn_b

  All big matmuls run in fp8e4 (e4m3) with MatmulPerfMode.DoubleRow (two
  contraction rows per pass).  counts are small ints => exact in fp8; the
  v-projection weights use an error-compensated hi+lo fp8 pair (plain fp8
  there costs ~2e-2 rel err; compensated is ~2.5e-3).  All de-scales fold
  into existing scale/bias operands (exp bias takes ln(s_e); the Z column
  carries the vq scale; a residual power of two rides on out_w).

  Sharding: data-parallel over B=32 instances, 4 per NeuronCore; params
  replicated.
"""

import math
from contextlib import ExitStack

import numpy as np
import ml_dtypes

import concourse.bass as bass
import concourse.mybir as mybir
import concourse.tile as tile
from concourse import bacc
from concourse.bass_utils import run_bass_kernel_spmd

# ---------------- problem constants (hardcoded) ----------------
D = 256
H = 8
P = 32
V = 257
EPS = 1e-5
B, C, L = 32, 2048, 8
VOC = 2 * V            # 514 combined (var, sign) literals
NCORES = 8
BPC = B // NCORES      # 4 instances per core
CB = C // 128          # 16 chunks of 128 clauses
NCC = 4                # clause column chunks of 512 for xT
hd = D // H

fp8 = mybir.dt.float8e4
fp16 = mybir.dt.float16
fp32 = mybir.dt.float32
AF = mybir.ActivationFunctionType
ALU = mybir.AluOpType
AX = mybir.AxisListType
DR = mybir.MatmulPerfMode.DoubleRow

E4NP = ml_dtypes.float8_e4m3
S_E = 32.0             # exp output scale (via +ln(S_E) on the bias)
ZV = 64.0              # Z-column constant (s_vq * 2^-kz)


def _emit(nc, tc, ctx, dr, out_dram):
    pc = ctx.enter_context(tc.tile_pool(name="consts", bufs=1))
    pcnt = ctx.enter_context(tc.tile_pool(name="cnt", bufs=2))
    px = ctx.enter_context(tc.tile_pool(name="x", bufs=2))
    px2 = ctx.enter_context(tc.tile_pool(name="x2", bufs=2))
    pexp = ctx.enter_context(tc.tile_pool(name="expv", bufs=2))
    pst = ctx.enter_context(tc.tile_pool(name="stats", bufs=2))
    psm = ctx.enter_context(tc.tile_pool(name="small", bufs=2))
    ps_x = ctx.enter_context(tc.tile_pool(name="ps_x", bufs=2, space="PSUM"))
    ps_sv = ctx.enter_context(tc.tile_pool(name="ps_sv", bufs=2, space="PSUM"))
    ps_st = ctx.enter_context(tc.tile_pool(name="ps_st", bufs=1, space="PSUM"))
    ps_bc = ctx.enter_context(tc.tile_pool(name="ps_bc", bufs=1, space="PSUM"))
    ps_tail = ctx.enter_context(tc.tile_pool(name="ps_tail", bufs=1, space="PSUM"))

    # ---- constants to SBUF ----
    tbl8 = pc.tile([128, 4, D], fp8, tag="tbl8")
    nc.sync.dma_start(out=tbl8[:], in_=dr["tbl8"][:])
    tbl2 = pc.tile([1, 2, D], fp8, tag="tbl2")
    nc.sync.dma_start(out=tbl2[:], in_=dr["tbl2"][:])
    wkv8 = pc.tile([128, 2, 3, D], fp8, tag="wkv8")
    nc.sync.dma_start(out=wkv8[:], in_=dr["wkv8"][:])
    owt = pc.tile([128, 2, D], fp16, tag="owt")
    nc.sync.dma_start(out=owt[:], in_=dr["owt"][:])
    pqb = pc.tile([P, D], fp32, tag="pqb")
    nc.sync.dma_start(out=pqb[:], in_=dr["pqb"][:])
    png = pc.tile([P, D], fp32, tag="png")
    nc.sync.dma_start(out=png[:], in_=dr["png"][:])
    pnb = pc.tile([P, D], fp32, tag="pnb")
    nc.sync.dma_start(out=pnb[:], in_=dr["pnb"][:])
    scl = pc.tile([128, 6], fp32, tag="scl")  # xsc, stu, stw, k2, 1/D, eps
    nc.sync.dma_start(out=scl[:], in_=dr["scl"][:])
    zc8 = pc.tile([128, CB], fp8, tag="zc8")
    nc.sync.dma_start(out=zc8[:], in_=dr["zc8"][:])
    ones16 = pc.tile([128, 1], fp16, tag="ones16")
    nc.sync.dma_start(out=ones16[:], in_=dr["ones16"][:])

    for b in range(BPC):
        # ---- load per-instance data (cnt chunked so compute starts early) ----
        cnt = pcnt.tile([128, 4, C], fp8, tag="cnt")
        for cc in range(NCC):
            nc.sync.dma_start(out=cnt[:, :, cc * 512:(cc + 1) * 512],
                              in_=dr["cnt8"][b, cc])
        cnt2 = pcnt.tile([1, 2, C], fp8, tag="cnt2")
        nc.sync.dma_start(out=cnt2[:], in_=dr["cnt2"][b])
        mb_t = psm.tile([128, CB], fp32, tag="mb")
        nc.sync.dma_start(out=mb_t[:], in_=dr["maskb"][b])

        # ---- xT[d%128, d//128, c] = tableT @ counts (fp8 DoubleRow) ----
        x8 = px.tile([128, 2, C], fp8, tag="x8")
        x2 = px2.tile([128, 2, C], fp16, tag="x2")
        for cc in range(NCC):
            csl = slice(cc * 512, (cc + 1) * 512)
            for dh in range(2):
                dsl = slice(dh * 128, (dh + 1) * 128)
                pxm = ps_x.tile([128, 512], fp32, tag="xmm")
                for pr in range(2):
                    nc.tensor.matmul(pxm[:],
                                     lhsT=tbl8[:, 2 * pr:2 * pr + 2, dsl],
                                     rhs=cnt[:, 2 * pr:2 * pr + 2, csl],
                                     start=(pr == 0), stop=False, perf_mode=DR)
                nc.tensor.matmul(pxm[:], lhsT=tbl2[:, :, dsl],
                                 rhs=cnt2[:, :, csl],
                                 start=False, stop=True, perf_mode=DR)
                nc.scalar.activation(x8[:, dh, csl], pxm[:], AF.Copy,
                                     scale=scl[:, 0:1])
                nc.gpsimd.tensor_mul(x2[:, dh, csl], x8[:, dh, csl],
                                     x8[:, dh, csl])

        # ---- stats: rs1[c] = rs_true/m ; rs2 = rs1*k2 ----
        pst_t = ps_st.tile([128, CB], fp32, tag="st")
        for cb in range(CB):
            for dh in range(2):
                nc.tensor.matmul(pst_t[:, cb:cb + 1],
                                 lhsT=x2[:, dh, cb * 128:(cb + 1) * 128],
                                 rhs=ones16[:],
                                 start=(dh == 0), stop=(dh == 1))
        sq = pst.tile([128, CB], fp32, tag="sq")
        nc.scalar.activation(sq[:], pst_t[:], AF.Sqrt,
                             bias=scl[:, 2:3], scale=scl[:, 1:2])
        rs1 = pst.tile([128, CB], fp32, tag="rs1")
        nc.vector.reciprocal(rs1[:], sq[:])
        rs2 = pst.tile([128, CB], fp32, tag="rs2")
        nc.vector.tensor_scalar_mul(rs2[:], rs1[:], scl[:, 3:4])

        # ---- s1v per clause chunk; exp / v-scale epilogues ----
        expT = pexp.tile([128, CB, 256], fp8, tag="expT")
        vqa = pexp.tile([128, CB, 257], fp8, tag="vqa")
        nc.sync.dma_start(out=vqa[:, :, 256:257], in_=dr["zc8"][:])
        for cb in range(CB):
            sv = ps_sv.tile([128, 512], fp32, tag="sv")
            lhsT = x8[:, :, cb * 128:(cb + 1) * 128]
            nc.tensor.matmul(sv[:], lhsT=lhsT, rhs=wkv8[:, :, 0:2, :],
                             start=True, stop=True, perf_mode=DR)
            nc.tensor.matmul(sv[:, 256:512], lhsT=lhsT, rhs=wkv8[:, :, 2, :],
                             start=False, stop=True, perf_mode=DR,
                             skip_group_check=True)
            nc.scalar.activation(expT[:, cb, :], sv[:, 0:256], AF.Exp,
                                 bias=mb_t[:, cb:cb + 1], scale=rs1[:, cb:cb + 1])
            nc.vector.tensor_scalar_mul(vqa[:, cb, 0:256], sv[:, 256:512],
                                        rs2[:, cb:cb + 1])

        # ---- bigctx[hp, he+1] = sum_c expT^T [vq | zc] (fp8 DoubleRow) ----
        pbc0 = ps_bc.tile([128, 257], fp32, tag="bc0")
        pbc1 = ps_bc.tile([128, 257], fp32, tag="bc1")
        pbc = [pbc0, pbc1]
        for hpc in range(2):
            for i in range(CB // 2):
                nc.tensor.matmul(pbc[hpc][:],
                                 lhsT=expT[:, 2 * i:2 * i + 2,
                                           hpc * 128:(hpc + 1) * 128],
                                 rhs=vqa[:, 2 * i:2 * i + 2, :],
                                 start=(i == 0), stop=(i == CB // 2 - 1),
                                 perf_mode=DR)

        # ---- ctx extraction: scale diag head blocks by 1/Z, transpose ----
        zr = pst.tile([128, 2], fp32, tag="zr")
        for hpc in range(2):
            nc.vector.reciprocal(zr[:, hpc:hpc + 1], pbc[hpc][:, 256:257])
        sc16 = psm.tile([128, 2, P], fp16, tag="sc16")
        ctxT = psm.tile([128, 2, P], fp16, tag="ctxT")
        for h in range(H):
            ch, o = divmod(h, 4)
            osl = slice(o * 32, o * 32 + 32)
            nc.vector.tensor_scalar_mul(sc16[osl, ch, :],
                                        pbc[ch][osl, h * 32:h * 32 + 32],
                                        zr[osl, ch:ch + 1])
            nc.vector.transpose(ctxT[osl, ch, :], sc16[osl, ch, :])

        # ---- refined = ctx @ out_w.T (fp16) ; + pqb ; LayerNorm ----
        prf_t = ps_tail.tile([P, D], fp32, tag="tail")
        for ch in range(2):
            nc.tensor.matmul(prf_t[:], lhsT=ctxT[:, ch, :], rhs=owt[:, ch, :],
                             start=(ch == 0), stop=(ch == 1))
        rfn = psm.tile([P, D], fp32, tag="rfn")
        nc.vector.tensor_add(rfn[:], prf_t[:], pqb[:])
        ssum = pst.tile([P, 1], fp32, tag="ssum")
        nc.vector.tensor_reduce(ssum[:], rfn[:], axis=AX.X, op=ALU.add)
        nm = pst.tile([P, 1], fp32, tag="nm")
        nc.vector.tensor_scalar_mul(nm[:], ssum[:], -1.0 / D)
        cen = psm.tile([P, D], fp32, tag="cen")
        nc.scalar.activation(cen[:], rfn[:], AF.Identity, bias=nm[:, 0:1])
        sq2 = psm.tile([P, D], fp32, tag="sq2")
        nc.vector.tensor_mul(sq2[:], cen[:], cen[:])
        vs = pst.tile([P, 1], fp32, tag="vs")
        nc.vector.tensor_reduce(vs[:], sq2[:], axis=AX.X, op=ALU.add)
        stdv = pst.tile([P, 1], fp32, tag="stdv")
        nc.scalar.activation(stdv[:], vs[:], AF.Sqrt,
                             bias=scl[0:P, 5:6], scale=scl[0:P, 4:5])
        rstd = pst.tile([P, 1], fp32, tag="rstd")
        nc.vector.reciprocal(rstd[:], stdv[:])
        t1 = psm.tile([P, D], fp32, tag="t1")
        nc.vector.scalar_tensor_tensor(t1[:], in0=cen[:], scalar=rstd[:, 0:1],
                                       in1=png[:], op0=ALU.mult, op1=ALU.mult)
        outt = psm.tile([P, D], fp32, tag="outt")
        nc.vector.tensor_add(outt[:], t1[:], pnb[:])
        nc.sync.dma_start(out=out_dram[b], in_=outt[:])


def _build_nc():
    nc = bacc.Bacc("TRN2", target_bir_lowering=False, debug=False,
                   num_devices=NCORES)
    dr = {}

    def din(name, shape, dt):
        dr[name] = nc.dram_tensor(name, shape, dt, kind="ExternalInput").ap()

    din("tbl8", [128, 4, D], fp8)
    din("tbl2", [1, 2, D], fp8)
    din("wkv8", [128, 2, 3, D], fp8)
    din("owt", [128, 2, D], fp16)
    din("pqb", [P, D], fp32)
    din("png", [P, D], fp32)
    din("pnb", [P, D], fp32)
    din("scl", [128, 6], fp32)
    din("zc8", [128, CB], fp8)
    din("ones16", [128, 1], fp16)
    din("cnt8", [BPC, NCC, 128, 4, 512], fp8)
    din("cnt2", [BPC, 1, 2, C], fp8)
    din("maskb", [BPC, 128, CB], fp32)
    out_dram = nc.dram_tensor("out", [BPC, P, D], fp32, kind="ExternalOutput").ap()

    with tile.TileContext(nc) as tc, ExitStack() as ctx:
        _emit(nc, tc, ctx, dr, out_dram)
    nc.compile()
    return nc


_NC_CACHE = None


def _get_nc():
    global _NC_CACHE
    if _NC_CACHE is None:
        _NC_CACHE = _build_nc()
    return _NC_CACHE


def _erf(x):
    try:
        from scipy.special import erf
        return erf(x)
    except Exception:
        from math import erf as _e
        return np.vectorize(_e)(x)


def _pow2(target, absmax):
    if absmax <= 0:
        return 1.0
    return 2.0 ** math.floor(math.log2(target / absmax))


def _e4(x):
    return np.asarray(x, np.float32).astype(E4NP)


def host_prepare(inputs):
    """Fold weights, quantize to fp8, build per-core input maps."""
    ve = inputs["var_embed"].astype(np.float64)
    se = inputs["sign_embed"].astype(np.float64)
    W1 = inputs["W1"].astype(np.float64)
    b1 = inputs["b1"].astype(np.float64)
    W2 = inputs["W2"].astype(np.float64)
    b2 = inputs["b2"].astype(np.float64)
    cn_g = inputs["cn_g"].astype(np.float64)
    cn_b = inputs["cn_b"].astype(np.float64)
    pq = inputs["prefix_queries"].astype(np.float64)
    in_w = inputs["in_proj_w"].astype(np.float64)
    in_b = inputs["in_proj_b"].astype(np.float64)
    out_w = inputs["out_w"].astype(np.float64)
    out_b = inputs["out_b"].astype(np.float64)
    pn_g = inputs["pn_g"].astype(np.float64)
    pn_b = inputs["pn_b"].astype(np.float64)

    # literal table over combined index j = v*2 + s; /L bakes the clause mean,
    # row-centering makes clause vectors exactly zero-mean under LN
    lit = np.concatenate([np.repeat(ve, 2, axis=0), np.tile(se, (V, 1))], axis=1)
    z = lit @ W1.T + b1
    gelu = 0.5 * z * (1.0 + _erf(z / math.sqrt(2.0)))
    table = (gelu @ W2.T + b2) / L
    table = table - table.mean(axis=1, keepdims=True)        # [514, D]

    s_t = _pow2(120.0, np.abs(table).max())
    ts = table * s_t
    tbl8 = np.ascontiguousarray(
        ts[:512].reshape(4, 128, D).transpose(1, 0, 2))
    tbl8 = _e4(tbl8)
    tbl2 = _e4(ts[512:514].reshape(1, 2, D))

    Wq, Wk, Wv = np.split(in_w, 3, axis=0)
    bq, bk, bv = np.split(in_b, 3)
    q = pq @ Wq.T + bq                                       # [P, D]
    scale = 1.0 / math.sqrt(hd)
    WkF = cn_g[:, None] * Wk.T
    WvF = cn_g[:, None] * Wv.T
    qh = q.reshape(P, H, hd)
    qbd = np.zeros((D, H * P))
    for h in range(H):
        qbd[h * hd:(h + 1) * hd, h * P:(h + 1) * P] = qh[:, h, :].T * scale
    WKQ = WkF @ qbd                                          # [D, 256]

    s_kq = _pow2(120.0, np.abs(WKQ).max())
    s_v = _pow2(120.0, np.abs(WvF).max())
    wkq8 = _e4(WKQ * s_kq)
    wv_hi = _e4(WvF * s_v)
    wv_lo = _e4(WvF * s_v - wv_hi.astype(np.float64))
    wkv8 = np.zeros((128, 2, 3, D), E4NP)
    for dh in range(2):
        dsl = slice(dh * 128, (dh + 1) * 128)
        wkv8[:, dh, 0, :] = wkq8[dsl]
        wkv8[:, dh, 1, :] = wv_hi[dsl]
        wkv8[:, dh, 2, :] = wv_lo[dsl]

    # x8 scale: bound on |x| from quantized-table row maxima over actual data
    tq = np.concatenate([
        tbl8.transpose(1, 0, 2).reshape(512, D).astype(np.float64),
        tbl2.reshape(2, D).astype(np.float64)])               # [514, D] * s_t
    rowmax = np.abs(tq).max(axis=1)                           # [514]
    ci = (inputs["var_idx"].astype(np.int64) * 2
          + inputs["sign_idx"].astype(np.int64))              # [B, C, L]
    bx = rowmax[ci].sum(axis=2).max()                         # scaled by s_t
    alpha = _pow2(120.0, bx)

    # vq scale from a Cauchy-Schwarz bound: |vq| <= sqrt(D)*max_col ||WvF||
    s_vq = _pow2(120.0, 16.0 * np.sqrt((WvF ** 2).sum(axis=0)).max())
    kz = s_vq / ZV                                            # folded into owt

    m = alpha * s_t * s_kq
    stu = s_kq * s_kq / D
    stw = EPS * m * m
    k2 = s_kq * s_vq / s_v
    scl = np.zeros((128, 6), np.float32)
    scl[:, 0] = alpha
    scl[:, 1] = stu
    scl[:, 2] = stw
    scl[:, 3] = k2
    scl[:, 4] = 1.0 / D
    scl[:, 5] = EPS

    owt = np.ascontiguousarray(
        (out_w.T / kz).reshape(2, 128, D).transpose(1, 0, 2)).astype(np.float16)
    bvF = cn_b @ Wv.T + bv                       # bk dropped (softmax shift)
    pqb = (pq + out_b + bvF @ out_w.T).astype(np.float32)
    png = np.broadcast_to(pn_g, (P, D)).astype(np.float32)
    pnb = np.broadcast_to(pn_b, (P, D)).astype(np.float32)
    zc8 = np.full((128, CB), ZV, E4NP)
    ones16 = np.ones((128, 1), np.float16)

    maskb_full = np.where(np.asarray(inputs["mask"]) > 0, 0.0, -1e9)
    lnse = math.log(S_E)

    in_maps = []
    for core in range(NCORES):
        cnt8 = np.zeros((BPC, NCC, 128, 4, 512), E4NP)
        cnt2 = np.zeros((BPC, 1, 2, C), E4NP)
        mkb = np.zeros((BPC, 128, CB), np.float32)
        for bl in range(BPC):
            bg = core * BPC + bl
            flat = ci[bg].reshape(-1)
            rows = np.repeat(np.arange(C, dtype=np.int64), L)
            cc = np.bincount(rows * 640 + flat,
                             minlength=C * 640).reshape(C, 640)
            full = cc[:, :512].reshape(C, 4, 128).transpose(2, 1, 0)  # [128,4,C]
            cnt8[bl] = _e4(full.reshape(128, 4, NCC, 512).transpose(
                2, 0, 1, 3))
            cnt2[bl, 0] = _e4(cc[:, 512:514].T)               # [2, C]
            mkb[bl] = maskb_full[bg].reshape(CB, 128).T + lnse
        in_maps.append({
            "tbl8": tbl8, "tbl2": tbl2, "wkv8": wkv8, "owt": owt,
            "pqb": pqb, "png": png, "pnb": pnb, "scl": scl, "zc8": zc8,
            "ones16": ones16, "cnt8": cnt8, "cnt2": cnt2, "maskb": mkb,
        })
    return in_maps


def kernel(**inputs):
    nc = _get_nc()
    in_maps = host_prepare(inputs)
    res = run_bass_kernel_spmd(nc, in_maps, core_ids=list(range(NCORES)))
    out = np.concatenate([res.results[i]["out"] for i in range(NCORES)], axis=0)
    return np.ascontiguousarray(out.astype(np.float32))


# revision 9
# speedup vs baseline: 1.4185x; 1.1843x over previous
"""Trainium2 Bass kernel for nn_CNFAdapter (fp8 DoubleRow, v3).

Algorithm (identical math to the reference, heavily folded):

  The literal MLP ``h = gelu([ve[v]; se[s]] @ W1.T + b1) @ W2.T + b2`` has
  only 514 distinct inputs (257 vars x 2 signs); it is folded on the host
  into a table ``T[514, 256]``.  Dividing T by L bakes in the clause mean;
  subtracting each row's d-mean makes clause vectors exactly zero-mean,
  removing the LN mean term.  Rows 512/513 are folded EXACTLY into the
  first 512 rows via least squares (T[512] = T[:512]^T w has exact
  solutions since rank(T[:512]) = 256), turning the histogram into a
  512-row matrix => the contraction is 4 clean 128-row chunks.

  Per instance (c = clause, d = hidden, hp = (head,query), he = (head,e)):
     xT[d, c]   = tableT @ counts'      (counts' = folded literal histogram)
     rs[c]      = 1/sqrt(sum_d x^2 / D + eps)      (DVE magic-rsqrt+Newton)
     sv[c, :]   = x @ [Wkq | WvF]       (Wkq folds cn_g, Wk, q, softmax scale)
     expT[c,hp] = exp(rs*sv_kq + maskbias)   (bk dropped: softmax shift inv.)
     vq[c, he]  = rs2 * sv_v
     bigctx[hp, he+1] = expT.T @ [vq | zconst]   (Z rides in the extra column)
     ctx[p, he] = diag head blocks / Z
     out        = LN(pqb + ctx @ out_w.T) * pn_g + pn_b

  All big matmuls are fp8e4 MatmulPerfMode.DoubleRow (measured 2x: a K=256
  DR matmul costs one K=128 fp16 matmul).  The v-projection weights use an
  error-compensated hi+lo fp8 pair (plain fp8 there is ~2e-2 rel err).
  De-scales fold into existing operands (exp bias takes ln(s_e), the Z
  column carries the vq scale, a residual power of two rides on out_w).
  The scalar engine runs only table-free Copy plus Exp (no activation-table
  thrash); sqrt/rsqrt run on the vector engine via bitcast magic-rsqrt.

  Sharding: data-parallel over B=32 instances, 4 per NeuronCore; params
  replicated.
"""

import math
from contextlib import ExitStack

import numpy as np
import ml_dtypes

import concourse.bass as bass
import concourse.mybir as mybir
import concourse.tile as tile
from concourse import bacc
from concourse.bass_utils import run_bass_kernel_spmd

# ---------------- problem constants (hardcoded) ----------------
D = 256
H = 8
P = 32
V = 257
EPS = 1e-5
B, C, L = 32, 2048, 8
NCORES = 8
BPC = B // NCORES      # 4 instances per core
CB = C // 128          # 16 chunks of 128 clauses
NCC = 4                # clause column chunks of 512 for xT
hd = D // H

fp8 = mybir.dt.float8e4
fp16 = mybir.dt.float16
fp32 = mybir.dt.float32
i32 = mybir.dt.int32
AF = mybir.ActivationFunctionType
ALU = mybir.AluOpType
AX = mybir.AxisListType
DR = mybir.MatmulPerfMode.DoubleRow

E4NP = ml_dtypes.float8_e4m3
S_E = 32.0             # exp output scale (via +ln(S_E) on the bias)
ZV = 64.0              # Z-column constant (s_vq * 2^-kz)
SQ8 = 2.0 ** -7        # x28 = (x8*SQ8)*x8, keeps squares in fp8 range
MAGIC = 0x5F3759DF


def _rsqrt(nc, pool, yv, n_part, nfree, tag):
    """rs = 1/sqrt(yv) on the vector engine: magic bitcast + 1 Newton."""
    r0i = pool.tile([n_part, nfree], i32, tag=tag + "i")
    nc.vector.tensor_scalar(r0i[:], yv.bitcast(i32), 1, None,
                            op0=ALU.arith_shift_right)
    r0j = pool.tile([n_part, nfree], i32, tag=tag + "j")
    nc.vector.tensor_scalar(r0j[:], r0i[:], -1, MAGIC,
                            op0=ALU.mult, op1=ALU.add)
    r0 = r0j[:].bitcast(fp32)
    t1 = pool.tile([n_part, nfree], fp32, tag=tag + "a")
    nc.vector.tensor_mul(t1[:], r0, r0)
    t2 = pool.tile([n_part, nfree], fp32, tag=tag + "b")
    nc.vector.tensor_mul(t2[:], t1[:], yv)
    t3 = pool.tile([n_part, nfree], fp32, tag=tag + "c")
    nc.vector.tensor_scalar(t3[:], t2[:], -0.5, 1.5, op0=ALU.mult, op1=ALU.add)
    rs = pool.tile([n_part, nfree], fp32, tag=tag + "r")
    nc.vector.tensor_mul(rs[:], r0, t3[:])
    return rs


def _emit(nc, tc, ctx, dr, out_dram):
    pc = ctx.enter_context(tc.tile_pool(name="consts", bufs=1))
    pcnt = ctx.enter_context(tc.tile_pool(name="cnt", bufs=2))
    px = ctx.enter_context(tc.tile_pool(name="x", bufs=2))
    px2 = ctx.enter_context(tc.tile_pool(name="x2", bufs=2))
    pexp = ctx.enter_context(tc.tile_pool(name="expv", bufs=2))
    pst = ctx.enter_context(tc.tile_pool(name="stats", bufs=2))
    psm = ctx.enter_context(tc.tile_pool(name="small", bufs=2))
    ps_x = ctx.enter_context(tc.tile_pool(name="ps_x", bufs=2, space="PSUM"))
    ps_sv = ctx.enter_context(tc.tile_pool(name="ps_sv", bufs=2, space="PSUM"))
    ps_st = ctx.enter_context(tc.tile_pool(name="ps_st", bufs=1, space="PSUM"))
    ps_bc = ctx.enter_context(tc.tile_pool(name="ps_bc", bufs=1, space="PSUM"))
    ps_tail = ctx.enter_context(tc.tile_pool(name="ps_tail", bufs=1, space="PSUM"))

    # ---- constants to SBUF ----
    tbl8 = pc.tile([128, 4, D], fp8, tag="tbl8")
    nc.sync.dma_start(out=tbl8[:], in_=dr["tbl8"][:])
    wkv8 = pc.tile([128, 2, 3, D], fp8, tag="wkv8")
    nc.sync.dma_start(out=wkv8[:], in_=dr["wkv8"][:])
    owt = pc.tile([128, 2, D], fp16, tag="owt")
    nc.sync.dma_start(out=owt[:], in_=dr["owt"][:])
    pqb = pc.tile([P, D], fp32, tag="pqb")
    nc.sync.dma_start(out=pqb[:], in_=dr["pqb"][:])
    png = pc.tile([P, D], fp32, tag="png")
    nc.sync.dma_start(out=png[:], in_=dr["png"][:])
    pnb = pc.tile([P, D], fp32, tag="pnb")
    nc.sync.dma_start(out=pnb[:], in_=dr["pnb"][:])
    scl = pc.tile([128, 6], fp32, tag="scl")  # xsc, stu, stw, k2, 1/D, eps
    nc.sync.dma_start(out=scl[:], in_=dr["scl"][:])
    zc8 = pc.tile([128, CB], fp8, tag="zc8")
    nc.sync.dma_start(out=zc8[:], in_=dr["zc8"][:])
    ones16 = pc.tile([128, 1], fp16, tag="ones16")
    nc.sync.dma_start(out=ones16[:], in_=dr["ones16"][:])

    for b in range(BPC):
        # ---- load per-instance data (cnt chunked so compute starts early) ----
        cnt = pcnt.tile([128, 4, C], fp8, tag="cnt")
        for cc in range(NCC):
            nc.sync.dma_start(out=cnt[:, :, cc * 512:(cc + 1) * 512],
                              in_=dr["cnt8"][b, cc])
        mb_t = psm.tile([128, CB], fp32, tag="mb")
        nc.sync.dma_start(out=mb_t[:], in_=dr["maskb"][b])

        # ---- xT[d%128, d//128, c] = tableT @ counts (fp8 DoubleRow) ----
        x8 = px.tile([128, 2, C], fp8, tag="x8")
        x2 = px2.tile([128, 2, C], fp16, tag="x2")
        for cc in range(NCC):
            csl = slice(cc * 512, (cc + 1) * 512)
            for dh in range(2):
                dsl = slice(dh * 128, (dh + 1) * 128)
                pxm = ps_x.tile([128, 512], fp32, tag="xmm")
                for pr in range(2):
                    nc.tensor.matmul(pxm[:],
                                     lhsT=tbl8[:, 2 * pr:2 * pr + 2, dsl],
                                     rhs=cnt[:, 2 * pr:2 * pr + 2, csl],
                                     start=(pr == 0), stop=(pr == 1),
                                     perf_mode=DR)
                if dh == 0:
                    nc.scalar.activation(x8[:, dh, csl], pxm[:], AF.Copy,
                                         scale=scl[:, 0:1])
                else:
                    nc.vector.tensor_scalar_mul(x8[:, dh, csl], pxm[:],
                                                scl[:, 0:1])
                nc.gpsimd.tensor_mul(x2[:, dh, csl], x8[:, dh, csl],
                                     x8[:, dh, csl])

        # ---- stats: ssq via fp8 DR matmul; rs1 = rsqrt on DVE ----
        pst_t = ps_st.tile([128, CB], fp32, tag="st")
        for cb in range(CB):
            for dh in range(2):
                nc.tensor.matmul(pst_t[:, cb:cb + 1],
                                 lhsT=x2[:, dh, cb * 128:(cb + 1) * 128],
                                 rhs=ones16[:],
                                 start=(dh == 0), stop=(dh == 1))
        yv = pst.tile([128, CB], fp32, tag="yv")
        nc.vector.tensor_scalar(yv[:], pst_t[:], scl[:, 1:2], scl[:, 2:3],
                                op0=ALU.mult, op1=ALU.add)
        rs1 = _rsqrt(nc, pst, yv[:], 128, CB, "rq")
        rs2 = pst.tile([128, CB], fp32, tag="rs2")
        nc.vector.tensor_scalar_mul(rs2[:], rs1[:], scl[:, 3:4])

        # ---- s1v per clause chunk; exp / v-scale epilogues; bigctx ----
        expT = pexp.tile([128, CB, 256], fp8, tag="expT")
        vqa = pexp.tile([128, CB, 257], fp8, tag="vqa")
        nc.sync.dma_start(out=vqa[:, :, 256:257], in_=dr["zc8"][:])
        pbc0 = ps_bc.tile([128, 257], fp32, tag="bc0")
        pbc1 = ps_bc.tile([128, 257], fp32, tag="bc1")
        pbc = [pbc0, pbc1]
        for cb in range(CB):
            sv = ps_sv.tile([128, 512], fp32, tag="sv")
            lhsT = x8[:, :, cb * 128:(cb + 1) * 128]
            nc.tensor.matmul(sv[:], lhsT=lhsT, rhs=wkv8[:, :, 0:2, :],
                             start=True, stop=True, perf_mode=DR)
            nc.tensor.matmul(sv[:, 256:512], lhsT=lhsT, rhs=wkv8[:, :, 2, :],
                             start=False, stop=True, perf_mode=DR,
                             skip_group_check=True)
            nc.scalar.activation(expT[:, cb, :], sv[:, 0:256], AF.Exp,
                                 bias=mb_t[:, cb:cb + 1], scale=rs1[:, cb:cb + 1])
            if cb % 2 == 0:
                nc.vector.tensor_scalar_mul(vqa[:, cb, 0:256], sv[:, 256:512],
                                            rs2[:, cb:cb + 1])
            else:
                nc.scalar.activation(vqa[:, cb, 0:256], sv[:, 256:512], AF.Copy,
                                     scale=rs2[:, cb:cb + 1])
            if cb % 2 == 1:
                i = cb // 2
                for hpc in range(2):
                    nc.tensor.matmul(pbc[hpc][:],
                                     lhsT=expT[:, cb - 1:cb + 1,
                                               hpc * 128:(hpc + 1) * 128],
                                     rhs=vqa[:, cb - 1:cb + 1, :],
                                     start=(i == 0), stop=(i == CB // 2 - 1),
                                     perf_mode=DR)

        # ---- ctx extraction: scale diag head blocks by 1/Z, transpose ----
        zr = pst.tile([128, 2], fp32, tag="zr")
        for hpc in range(2):
            nc.vector.reciprocal(zr[:, hpc:hpc + 1], pbc[hpc][:, 256:257])
        sc16 = psm.tile([128, 2, P], fp16, tag="sc16")
        ctxT = psm.tile([128, 2, P], fp16, tag="ctxT")
        for h in range(H):
            ch, o = divmod(h, 4)
            osl = slice(o * 32, o * 32 + 32)
            nc.vector.tensor_scalar_mul(sc16[osl, ch, :],
                                        pbc[ch][osl, h * 32:h * 32 + 32],
                                        zr[osl, ch:ch + 1])
            nc.vector.transpose(ctxT[osl, ch, :], sc16[osl, ch, :])

        # ---- refined = ctx @ out_w.T (fp16) ; + pqb ; LayerNorm ----
        prf_t = ps_tail.tile([P, D], fp32, tag="tail")
        for ch in range(2):
            nc.tensor.matmul(prf_t[:], lhsT=ctxT[:, ch, :], rhs=owt[:, ch, :],
                             start=(ch == 0), stop=(ch == 1))
        rfn = psm.tile([P, D], fp32, tag="rfn")
        nc.vector.tensor_add(rfn[:], prf_t[:], pqb[:])
        ssum = pst.tile([P, 1], fp32, tag="ssum")
        nc.vector.tensor_reduce(ssum[:], rfn[:], axis=AX.X, op=ALU.add)
        nm = pst.tile([P, 1], fp32, tag="nm")
        nc.vector.tensor_scalar_mul(nm[:], ssum[:], -1.0 / D)
        cen = psm.tile([P, D], fp32, tag="cen")
        nc.vector.tensor_scalar_add(cen[:], rfn[:], nm[:, 0:1])
        sq2 = psm.tile([P, D], fp32, tag="sq2")
        nc.vector.tensor_mul(sq2[:], cen[:], cen[:])
        vs = pst.tile([P, 1], fp32, tag="vs")
        nc.vector.tensor_reduce(vs[:], sq2[:], axis=AX.X, op=ALU.add)
        yt = pst.tile([P, 1], fp32, tag="yt")
        nc.vector.tensor_scalar(yt[:], vs[:], 1.0 / D, EPS,
                                op0=ALU.mult, op1=ALU.add)
        rstd = _rsqrt(nc, pst, yt[:], P, 1, "rt")
        t1 = psm.tile([P, D], fp32, tag="t1")
        nc.vector.scalar_tensor_tensor(t1[:], in0=cen[:], scalar=rstd[:, 0:1],
                                       in1=png[:], op0=ALU.mult, op1=ALU.mult)
        outt = psm.tile([P, D], fp32, tag="outt")
        nc.vector.tensor_add(outt[:], t1[:], pnb[:])
        nc.sync.dma_start(out=out_dram[b], in_=outt[:])


def _build_nc():
    nc = bacc.Bacc("TRN2", target_bir_lowering=False, debug=False,
                   num_devices=NCORES)
    dr = {}

    def din(name, shape, dt):
        dr[name] = nc.dram_tensor(name, shape, dt, kind="ExternalInput").ap()

    din("tbl8", [128, 4, D], fp8)
    din("wkv8", [128, 2, 3, D], fp8)
    din("owt", [128, 2, D], fp16)
    din("pqb", [P, D], fp32)
    din("png", [P, D], fp32)
    din("pnb", [P, D], fp32)
    din("scl", [128, 6], fp32)
    din("zc8", [128, CB], fp8)
    din("ones16", [128, 1], fp16)
    din("cnt8", [BPC, NCC, 128, 4, 512], fp8)
    din("maskb", [BPC, 128, CB], fp32)
    out_dram = nc.dram_tensor("out", [BPC, P, D], fp32, kind="ExternalOutput").ap()

    with tile.TileContext(nc) as tc, ExitStack() as ctx:
        _emit(nc, tc, ctx, dr, out_dram)
    nc.compile()
    return nc


_NC_CACHE = None


def _get_nc():
    global _NC_CACHE
    if _NC_CACHE is None:
        _NC_CACHE = _build_nc()
    return _NC_CACHE


def _erf(x):
    try:
        from scipy.special import erf
        return erf(x)
    except Exception:
        from math import erf as _e
        return np.vectorize(_e)(x)


def _pow2(target, absmax):
    if absmax <= 0:
        return 1.0
    return 2.0 ** math.floor(math.log2(target / absmax))


def _e4(x):
    return np.asarray(x, np.float32).astype(E4NP)


def host_prepare(inputs):
    """Fold weights, quantize to fp8, build per-core input maps."""
    ve = inputs["var_embed"].astype(np.float64)
    se = inputs["sign_embed"].astype(np.float64)
    W1 = inputs["W1"].astype(np.float64)
    b1 = inputs["b1"].astype(np.float64)
    W2 = inputs["W2"].astype(np.float64)
    b2 = inputs["b2"].astype(np.float64)
    cn_g = inputs["cn_g"].astype(np.float64)
    cn_b = inputs["cn_b"].astype(np.float64)
    pq = inputs["prefix_queries"].astype(np.float64)
    in_w = inputs["in_proj_w"].astype(np.float64)
    in_b = inputs["in_proj_b"].astype(np.float64)
    out_w = inputs["out_w"].astype(np.float64)
    out_b = inputs["out_b"].astype(np.float64)
    pn_g = inputs["pn_g"].astype(np.float64)
    pn_b = inputs["pn_b"].astype(np.float64)

    # literal table over combined index j = v*2 + s; /L bakes the clause mean,
    # row-centering makes clause vectors exactly zero-mean under LN
    lit = np.concatenate([np.repeat(ve, 2, axis=0), np.tile(se, (V, 1))], axis=1)
    z = lit @ W1.T + b1
    gelu = 0.5 * z * (1.0 + _erf(z / math.sqrt(2.0)))
    table = (gelu @ W2.T + b2) / L
    table = table - table.mean(axis=1, keepdims=True)        # [514, D]

    # fold rows 512/513 exactly into the first 512 (min-norm least squares)
    A = table[:512].T                                        # [256, 512]
    w1f, *_ = np.linalg.lstsq(A, table[512], rcond=None)
    w2f, *_ = np.linalg.lstsq(A, table[513], rcond=None)

    s_t = _pow2(120.0, np.abs(table[:512]).max())
    ts = table[:512] * s_t
    tbl8 = _e4(np.ascontiguousarray(ts.reshape(4, 128, D).transpose(1, 0, 2)))

    Wq, Wk, Wv = np.split(in_w, 3, axis=0)
    bq, bk, bv = np.split(in_b, 3)
    q = pq @ Wq.T + bq                                       # [P, D]
    scale = 1.0 / math.sqrt(hd)
    WkF = cn_g[:, None] * Wk.T
    WvF = cn_g[:, None] * Wv.T
    qh = q.reshape(P, H, hd)
    qbd = np.zeros((D, H * P))
    for h in range(H):
        qbd[h * hd:(h + 1) * hd, h * P:(h + 1) * P] = qh[:, h, :].T * scale
    WKQ = WkF @ qbd                                          # [D, 256]

    s_kq = _pow2(120.0, np.abs(WKQ).max())
    s_v = _pow2(120.0, np.abs(WvF).max())
    wkq8 = _e4(WKQ * s_kq)
    wv_hi = _e4(WvF * s_v)
    wv_lo = _e4(WvF * s_v - wv_hi.astype(np.float64))
    wkv8 = np.zeros((128, 2, 3, D), E4NP)
    for dh in range(2):
        dsl = slice(dh * 128, (dh + 1) * 128)
        wkv8[:, dh, 0, :] = wkq8[dsl]
        wkv8[:, dh, 1, :] = wv_hi[dsl]
        wkv8[:, dh, 2, :] = wv_lo[dsl]

    # x8 scale from a per-clause bound on |x| (g extends rowmax to the fold)
    tq = tbl8.transpose(1, 0, 2).reshape(512, D).astype(np.float64)  # *s_t
    rowmax = np.abs(tq).max(axis=1)                           # [512]
    g = np.concatenate([rowmax,
                        [np.abs(w1f) @ rowmax, np.abs(w2f) @ rowmax]])
    ci = (inputs["var_idx"].astype(np.int64) * 2
          + inputs["sign_idx"].astype(np.int64))              # [B, C, L]
    bx = g[ci].sum(axis=2).max()
    alpha = _pow2(120.0, bx)

    # vq scale from a Cauchy-Schwarz bound: |vq| <= sqrt(D)*max_col ||WvF||
    s_vq = _pow2(120.0, 16.0 * np.sqrt((WvF ** 2).sum(axis=0)).max())
    kz = s_vq / ZV                                            # folded into owt

    m = alpha * s_t * s_kq
    scl = np.zeros((128, 6), np.float32)
    scl[:, 0] = alpha
    scl[:, 1] = s_kq * s_kq / D           # ssq_meas = (alpha*s_t)^2*ssq
    scl[:, 2] = EPS * m * m
    scl[:, 3] = s_kq * s_vq / s_v
    scl[:, 4] = 1.0 / D
    scl[:, 5] = EPS

    owt = np.ascontiguousarray(
        (out_w.T / kz).reshape(2, 128, D).transpose(1, 0, 2)).astype(np.float16)
    bvF = cn_b @ Wv.T + bv                       # bk dropped (softmax shift)
    pqb = (pq + out_b + bvF @ out_w.T).astype(np.float32)
    png = np.broadcast_to(pn_g, (P, D)).astype(np.float32)
    pnb = np.broadcast_to(pn_b, (P, D)).astype(np.float32)
    zc8 = np.full((128, CB), ZV, E4NP)
    ones16 = np.ones((128, 1), np.float16)

    maskb_full = np.where(np.asarray(inputs["mask"]) > 0, 0.0, -1e9)
    lnse = math.log(S_E)

    in_maps = []
    for core in range(NCORES):
        cnt8 = np.zeros((BPC, NCC, 128, 4, 512), E4NP)
        mkb = np.zeros((BPC, 128, CB), np.float32)
        for bl in range(BPC):
            bg = core * BPC + bl
            flat = ci[bg].reshape(-1)
            rows = np.repeat(np.arange(C, dtype=np.int64), L)
            cc = np.bincount(rows * 640 + flat,
                             minlength=C * 640).reshape(C, 640).astype(np.float64)
            full = cc[:, :512] + np.outer(cc[:, 512], w1f) + np.outer(cc[:, 513], w2f)
            full = full.T.reshape(4, 128, C).transpose(1, 0, 2)   # [128, 4, C]
            cnt8[bl] = _e4(full.reshape(128, 4, NCC, 512).transpose(2, 0, 1, 3))
            mkb[bl] = maskb_full[bg].reshape(CB, 128).T + lnse
        in_maps.append({
            "tbl8": tbl8, "wkv8": wkv8, "owt": owt,
            "pqb": pqb, "png": png, "pnb": pnb, "scl": scl, "zc8": zc8,
            "ones16": ones16, "cnt8": cnt8, "maskb": mkb,
        })
    return in_maps


def kernel(**inputs):
    nc = _get_nc()
    in_maps = host_prepare(inputs)
    res = run_bass_kernel_spmd(nc, in_maps, core_ids=list(range(NCORES)))
    out = np.concatenate([res.results[i]["out"] for i in range(NCORES)], axis=0)
    return np.ascontiguousarray(out.astype(np.float32))


# revision 11
# speedup vs baseline: 1.5744x; 1.1099x over previous
"""Trainium2 Bass kernel for nn_CNFAdapter (fp8 DoubleRow, v3).

Algorithm (identical math to the reference, heavily folded):

  The literal MLP ``h = gelu([ve[v]; se[s]] @ W1.T + b1) @ W2.T + b2`` has
  only 514 distinct inputs (257 vars x 2 signs); it is folded on the host
  into a table ``T[514, 256]``.  Dividing T by L bakes in the clause mean;
  subtracting each row's d-mean makes clause vectors exactly zero-mean,
  removing the LN mean term.  Rows 512/513 are folded EXACTLY into the
  first 512 rows via least squares (T[512] = T[:512]^T w has exact
  solutions since rank(T[:512]) = 256), turning the histogram into a
  512-row matrix => the contraction is 4 clean 128-row chunks.

  Per instance (c = clause, d = hidden, hp = (head,query), he = (head,e)):
     xT[d, c]   = tableT @ counts'      (counts' = folded literal histogram)
     rs[c]      = 1/sqrt(sum_d x^2 / D + eps)      (DVE magic-rsqrt+Newton)
     sv[c, :]   = x @ [Wkq | WvF]       (Wkq folds cn_g, Wk, q, softmax scale)
     expT[c,hp] = exp(rs*sv_kq + maskbias)   (bk dropped: softmax shift inv.)
     vq[c, he]  = rs2 * sv_v
     bigctx[hp, he+1] = expT.T @ [vq | zconst]   (Z rides in the extra column)
     ctx[p, he] = diag head blocks / Z
     out        = LN(pqb + ctx @ out_w.T) * pn_g + pn_b

  All big matmuls are fp8e4 MatmulPerfMode.DoubleRow (measured 2x: a K=256
  DR matmul costs one K=128 fp16 matmul).  The v-projection weights use an
  error-compensated hi+lo fp8 pair (plain fp8 there is ~2e-2 rel err).
  De-scales fold into existing operands (exp bias takes ln(s_e), the Z
  column carries the vq scale, a residual power of two rides on out_w).
  The scalar engine runs only table-free Copy plus Exp (no activation-table
  thrash); sqrt/rsqrt run on the vector engine via bitcast magic-rsqrt.

  Sharding: data-parallel over B=32 instances, 4 per NeuronCore; params
  replicated.
"""

import math
from contextlib import ExitStack

import numpy as np
import ml_dtypes

import concourse.bass as bass
import concourse.mybir as mybir
import concourse.tile as tile
from concourse import bacc
from concourse.bass_utils import run_bass_kernel_spmd

# ---------------- problem constants (hardcoded) ----------------
D = 256
H = 8
P = 32
V = 257
EPS = 1e-5
B, C, L = 32, 2048, 8
NCORES = 8
BPC = B // NCORES      # 4 instances per core
CB = C // 128          # 16 chunks of 128 clauses
NCC = 4                # clause column chunks of 512 for xT
hd = D // H

fp8 = mybir.dt.float8e4
fp8e5 = mybir.dt.float8e5
fp16 = mybir.dt.float16
fp32 = mybir.dt.float32
i32 = mybir.dt.int32
AF = mybir.ActivationFunctionType
ALU = mybir.AluOpType
AX = mybir.AxisListType
DR = mybir.MatmulPerfMode.DoubleRow

E4NP = ml_dtypes.float8_e4m3
E5NP = ml_dtypes.float8_e5m2
S_E = 32.0             # exp output scale (via +ln(S_E) on the bias)
ZV = 64.0              # Z-column constant (s_vq * 2^-kz)
SQ8 = 2.0 ** -7        # x28 = (x8*SQ8)*x8, keeps squares in fp8 range
MAGIC = 0x5F3759DF


def _rsqrt(nc, pool, yv, n_part, nfree, tag):
    """rs = 1/sqrt(yv) on the vector engine: magic bitcast + 1 Newton."""
    r0i = pool.tile([n_part, nfree], i32, tag=tag + "i")
    nc.vector.tensor_scalar(r0i[:], yv.bitcast(i32), 1, None,
                            op0=ALU.arith_shift_right)
    r0j = pool.tile([n_part, nfree], i32, tag=tag + "j")
    nc.vector.tensor_scalar(r0j[:], r0i[:], -1, MAGIC,
                            op0=ALU.mult, op1=ALU.add)
    r0 = r0j[:].bitcast(fp32)
    t1 = pool.tile([n_part, nfree], fp32, tag=tag + "a")
    nc.vector.tensor_mul(t1[:], r0, r0)
    t2 = pool.tile([n_part, nfree], fp32, tag=tag + "b")
    nc.vector.tensor_mul(t2[:], t1[:], yv)
    t3 = pool.tile([n_part, nfree], fp32, tag=tag + "c")
    nc.vector.tensor_scalar(t3[:], t2[:], -0.5, 1.5, op0=ALU.mult, op1=ALU.add)
    rs = pool.tile([n_part, nfree], fp32, tag=tag + "r")
    nc.vector.tensor_mul(rs[:], r0, t3[:])
    return rs


def _emit(nc, tc, ctx, dr, out_dram):
    pc = ctx.enter_context(tc.tile_pool(name="consts", bufs=1))
    pcnt = ctx.enter_context(tc.tile_pool(name="cnt", bufs=2))
    px = ctx.enter_context(tc.tile_pool(name="x", bufs=2))
    px2 = ctx.enter_context(tc.tile_pool(name="x2", bufs=2))
    pexp = ctx.enter_context(tc.tile_pool(name="expv", bufs=2))
    pst = ctx.enter_context(tc.tile_pool(name="stats", bufs=2))
    psm = ctx.enter_context(tc.tile_pool(name="small", bufs=2))
    ps_mm = ctx.enter_context(tc.tile_pool(name="ps_mm", bufs=4, space="PSUM"))
    ps_bc = ctx.enter_context(tc.tile_pool(name="ps_bc", bufs=1, space="PSUM"))
    ps_stail = ctx.enter_context(tc.tile_pool(name="ps_stail", bufs=1, space="PSUM"))

    # ---- constants to SBUF ----
    tbl8 = pc.tile([128, 4, D], fp8, tag="tbl8")
    nc.sync.dma_start(out=tbl8[:], in_=dr["tbl8"][:])
    wkv8 = pc.tile([128, 2, 3, D], fp8, tag="wkv8")
    nc.sync.dma_start(out=wkv8[:], in_=dr["wkv8"][:])
    owt = pc.tile([128, 2, D], fp16, tag="owt")
    nc.sync.dma_start(out=owt[:], in_=dr["owt"][:])
    pqb = pc.tile([P, D], fp32, tag="pqb")
    nc.sync.dma_start(out=pqb[:], in_=dr["pqb"][:])
    png = pc.tile([P, D], fp32, tag="png")
    nc.sync.dma_start(out=png[:], in_=dr["png"][:])
    pnb = pc.tile([P, D], fp32, tag="pnb")
    nc.sync.dma_start(out=pnb[:], in_=dr["pnb"][:])
    scl = pc.tile([128, 6], fp32, tag="scl")  # xsc, stu, stw, k2, 1/D, eps
    nc.sync.dma_start(out=scl[:], in_=dr["scl"][:])
    zc8 = pc.tile([128, CB], fp8, tag="zc8")
    nc.sync.dma_start(out=zc8[:], in_=dr["zc8"][:])
    ones8 = pc.tile([128, 2, 1], fp8e5, tag="ones8")
    nc.sync.dma_start(out=ones8[:], in_=dr["ones8"][:])

    for b in range(BPC):
        # ---- load per-instance data (cnt chunked so compute starts early) ----
        cnt = pcnt.tile([128, 4, C], fp8, tag="cnt")
        for cc in range(NCC):
            eng = nc.sync if cc % 2 == 0 else nc.scalar
            eng.dma_start(out=cnt[:, :, cc * 512:(cc + 1) * 512],
                          in_=dr["cnt8"][b, cc])
        mb_t = psm.tile([128, CB], fp32, tag="mb")
        nc.sync.dma_start(out=mb_t[:], in_=dr["maskb"][b])

        # ---- xT[d%128, d//128, c] = tableT @ counts (fp8 DoubleRow) ----
        x8 = px.tile([128, 2, C], fp8, tag="x8")
        x2 = px2.tile([128, 2, C], fp8e5, tag="x2")
        pst_t = ps_stail.tile([128, CB], fp32, tag="st")
        for cc in range(NCC):
            csl = slice(cc * 512, (cc + 1) * 512)
            for dh in range(2):
                dsl = slice(dh * 128, (dh + 1) * 128)
                pxm = ps_mm.tile([128, 512], fp32, tag="mm")
                for pr in range(2):
                    nc.tensor.matmul(pxm[:],
                                     lhsT=tbl8[:, 2 * pr:2 * pr + 2, dsl],
                                     rhs=cnt[:, 2 * pr:2 * pr + 2, csl],
                                     start=(pr == 0), stop=(pr == 1),
                                     perf_mode=DR)
                if dh == 0:
                    nc.scalar.activation(x8[:, dh, csl], pxm[:], AF.Copy,
                                         scale=scl[:, 0:1])
                else:
                    nc.vector.tensor_scalar_mul(x8[:, dh, csl], pxm[:],
                                                scl[:, 0:1])
                nc.gpsimd.tensor_mul(x2[:, dh, csl], x8[:, dh, csl],
                                     x8[:, dh, csl])
            for cb in range(4 * cc, 4 * cc + 4):
                nc.tensor.matmul(pst_t[:, cb:cb + 1],
                                 lhsT=x2[:, :, cb * 128:(cb + 1) * 128],
                                 rhs=ones8[:], start=True, stop=True,
                                 perf_mode=DR)
        yv = pst.tile([128, CB], fp32, tag="yv")
        nc.vector.tensor_scalar(yv[:], pst_t[:], scl[:, 1:2], scl[:, 2:3],
                                op0=ALU.mult, op1=ALU.add)
        rs1 = _rsqrt(nc, pst, yv[:], 128, CB, "rq")
        rs2 = pst.tile([128, CB], fp32, tag="rs2")
        nc.vector.tensor_scalar_mul(rs2[:], rs1[:], scl[:, 3:4])

        # ---- s1v per clause chunk; exp / v-scale epilogues; bigctx ----
        expT = pexp.tile([128, CB, 256], fp8, tag="expT")
        vqa = pexp.tile([128, CB, 257], fp8, tag="vqa")
        nc.sync.dma_start(out=vqa[:, :, 256:257], in_=dr["zc8"][:])
        pbc0 = ps_bc.tile([128, 257], fp32, tag="bc0")
        pbc1 = ps_bc.tile([128, 257], fp32, tag="bc1")
        pbc = [pbc0, pbc1]
        for cb in range(CB):
            sv = ps_mm.tile([128, 512], fp32, tag="mm")
            lhsT = x8[:, :, cb * 128:(cb + 1) * 128]
            nc.tensor.matmul(sv[:], lhsT=lhsT, rhs=wkv8[:, :, 0:2, :],
                             start=True, stop=True, perf_mode=DR)
            nc.tensor.matmul(sv[:, 256:512], lhsT=lhsT, rhs=wkv8[:, :, 2, :],
                             start=False, stop=True, perf_mode=DR,
                             skip_group_check=True)
            nc.scalar.activation(expT[:, cb, :], sv[:, 0:256], AF.Exp,
                                 bias=mb_t[:, cb:cb + 1], scale=rs1[:, cb:cb + 1])
            nc.vector.tensor_scalar_mul(vqa[:, cb, 0:256], sv[:, 256:512],
                                        rs2[:, cb:cb + 1])
            if cb % 2 == 1:
                i = cb // 2
                for hpc in range(2):
                    nc.tensor.matmul(pbc[hpc][:],
                                     lhsT=expT[:, cb - 1:cb + 1,
                                               hpc * 128:(hpc + 1) * 128],
                                     rhs=vqa[:, cb - 1:cb + 1, :],
                                     start=(i == 0), stop=(i == CB // 2 - 1),
                                     perf_mode=DR)

        # ---- ctx extraction: scale diag head blocks by 1/Z, transpose ----
        zr = pst.tile([128, 2], fp32, tag="zr")
        for hpc in range(2):
            nc.vector.reciprocal(zr[:, hpc:hpc + 1], pbc[hpc][:, 256:257])
        sc16 = psm.tile([128, 2, P], fp16, tag="sc16")
        ctxT = psm.tile([128, 2, P], fp16, tag="ctxT")
        for h in range(H):
            ch, o = divmod(h, 4)
            osl = slice(o * 32, o * 32 + 32)
            nc.vector.tensor_scalar_mul(sc16[osl, ch, :],
                                        pbc[ch][osl, h * 32:h * 32 + 32],
                                        zr[osl, ch:ch + 1])
        for o in range(4):
            osl = slice(o * 32, o * 32 + 32)
            nc.vector.transpose(ctxT[osl, :, :].rearrange("p a b -> p (a b)"),
                                sc16[osl, :, :].rearrange("p a b -> p (a b)"))

        # ---- refined = ctx @ out_w.T (fp16) ; + pqb ; LayerNorm ----
        prf_t = ps_stail.tile([P, D], fp32, tag="tail")
        for ch in range(2):
            nc.tensor.matmul(prf_t[:], lhsT=ctxT[:, ch, :], rhs=owt[:, ch, :],
                             start=(ch == 0), stop=(ch == 1))
        rfn = psm.tile([P, D], fp32, tag="rfn")
        nc.vector.tensor_add(rfn[:], prf_t[:], pqb[:])
        ssum = pst.tile([P, 1], fp32, tag="ssum")
        nc.vector.tensor_reduce(ssum[:], rfn[:], axis=AX.X, op=ALU.add)
        nm = pst.tile([P, 1], fp32, tag="nm")
        nc.vector.tensor_scalar_mul(nm[:], ssum[:], -1.0 / D)
        cen = psm.tile([P, D], fp32, tag="cen")
        nc.vector.tensor_scalar_add(cen[:], rfn[:], nm[:, 0:1])
        sq2 = psm.tile([P, D], fp32, tag="sq2")
        nc.vector.tensor_mul(sq2[:], cen[:], cen[:])
        vs = pst.tile([P, 1], fp32, tag="vs")
        nc.vector.tensor_reduce(vs[:], sq2[:], axis=AX.X, op=ALU.add)
        yt = pst.tile([P, 1], fp32, tag="yt")
        nc.vector.tensor_scalar(yt[:], vs[:], 1.0 / D, EPS,
                                op0=ALU.mult, op1=ALU.add)
        rstd = _rsqrt(nc, pst, yt[:], P, 1, "rt")
        t1 = psm.tile([P, D], fp32, tag="t1")
        nc.vector.scalar_tensor_tensor(t1[:], in0=cen[:], scalar=rstd[:, 0:1],
                                       in1=png[:], op0=ALU.mult, op1=ALU.mult)
        outt = psm.tile([P, D], fp32, tag="outt")
        nc.vector.tensor_add(outt[:], t1[:], pnb[:])
        nc.sync.dma_start(out=out_dram[b], in_=outt[:])


def _build_nc():
    nc = bacc.Bacc("TRN2", target_bir_lowering=False, debug=False,
                   num_devices=NCORES)
    dr = {}

    def din(name, shape, dt):
        dr[name] = nc.dram_tensor(name, shape, dt, kind="ExternalInput").ap()

    din("tbl8", [128, 4, D], fp8)
    din("wkv8", [128, 2, 3, D], fp8)
    din("owt", [128, 2, D], fp16)
    din("pqb", [P, D], fp32)
    din("png", [P, D], fp32)
    din("pnb", [P, D], fp32)
    din("scl", [128, 6], fp32)
    din("zc8", [128, CB], fp8)
    din("ones8", [128, 2, 1], fp8e5)
    din("cnt8", [BPC, NCC, 128, 4, 512], fp8)
    din("maskb", [BPC, 128, CB], fp32)
    out_dram = nc.dram_tensor("out", [BPC, P, D], fp32, kind="ExternalOutput").ap()

    with tile.TileContext(nc) as tc, ExitStack() as ctx:
        _emit(nc, tc, ctx, dr, out_dram)
    nc.compile()
    return nc


_NC_CACHE = None


def _get_nc():
    global _NC_CACHE
    if _NC_CACHE is None:
        _NC_CACHE = _build_nc()
    return _NC_CACHE


def _erf(x):
    try:
        from scipy.special import erf
        return erf(x)
    except Exception:
        from math import erf as _e
        return np.vectorize(_e)(x)


def _pow2(target, absmax):
    if absmax <= 0:
        return 1.0
    return 2.0 ** math.floor(math.log2(target / absmax))


def _e4(x):
    return np.asarray(x, np.float32).astype(E4NP)


def host_prepare(inputs):
    """Fold weights, quantize to fp8, build per-core input maps."""
    ve = inputs["var_embed"].astype(np.float64)
    se = inputs["sign_embed"].astype(np.float64)
    W1 = inputs["W1"].astype(np.float64)
    b1 = inputs["b1"].astype(np.float64)
    W2 = inputs["W2"].astype(np.float64)
    b2 = inputs["b2"].astype(np.float64)
    cn_g = inputs["cn_g"].astype(np.float64)
    cn_b = inputs["cn_b"].astype(np.float64)
    pq = inputs["prefix_queries"].astype(np.float64)
    in_w = inputs["in_proj_w"].astype(np.float64)
    in_b = inputs["in_proj_b"].astype(np.float64)
    out_w = inputs["out_w"].astype(np.float64)
    out_b = inputs["out_b"].astype(np.float64)
    pn_g = inputs["pn_g"].astype(np.float64)
    pn_b = inputs["pn_b"].astype(np.float64)

    # literal table over combined index j = v*2 + s; /L bakes the clause mean,
    # row-centering makes clause vectors exactly zero-mean under LN
    lit = np.concatenate([np.repeat(ve, 2, axis=0), np.tile(se, (V, 1))], axis=1)
    z = lit @ W1.T + b1
    gelu = 0.5 * z * (1.0 + _erf(z / math.sqrt(2.0)))
    table = (gelu @ W2.T + b2) / L
    table = table - table.mean(axis=1, keepdims=True)        # [514, D]

    # fold rows 512/513 exactly into the first 512 (min-norm least squares)
    A = table[:512].T                                        # [256, 512]
    w1f, *_ = np.linalg.lstsq(A, table[512], rcond=None)
    w2f, *_ = np.linalg.lstsq(A, table[513], rcond=None)

    s_t = _pow2(120.0, np.abs(table[:512]).max())
    ts = table[:512] * s_t
    tbl8 = _e4(np.ascontiguousarray(ts.reshape(4, 128, D).transpose(1, 0, 2)))

    Wq, Wk, Wv = np.split(in_w, 3, axis=0)
    bq, bk, bv = np.split(in_b, 3)
    q = pq @ Wq.T + bq                                       # [P, D]
    scale = 1.0 / math.sqrt(hd)
    WkF = cn_g[:, None] * Wk.T
    WvF = cn_g[:, None] * Wv.T
    qh = q.reshape(P, H, hd)
    qbd = np.zeros((D, H * P))
    for h in range(H):
        qbd[h * hd:(h + 1) * hd, h * P:(h + 1) * P] = qh[:, h, :].T * scale
    WKQ = WkF @ qbd                                          # [D, 256]

    s_kq = _pow2(120.0, np.abs(WKQ).max())
    s_v = _pow2(120.0, np.abs(WvF).max())
    wkq8 = _e4(WKQ * s_kq)
    wv_hi = _e4(WvF * s_v)
    wv_lo = _e4(WvF * s_v - wv_hi.astype(np.float64))
    wkv8 = np.zeros((128, 2, 3, D), E4NP)
    for dh in range(2):
        dsl = slice(dh * 128, (dh + 1) * 128)
        wkv8[:, dh, 0, :] = wkq8[dsl]
        wkv8[:, dh, 1, :] = wv_hi[dsl]
        wkv8[:, dh, 2, :] = wv_lo[dsl]

    # x8 scale from a per-clause bound on |x| (g extends rowmax to the fold)
    tq = tbl8.transpose(1, 0, 2).reshape(512, D).astype(np.float64)  # *s_t
    rowmax = np.abs(tq).max(axis=1)                           # [512]
    g = np.concatenate([rowmax,
                        [np.abs(w1f) @ rowmax, np.abs(w2f) @ rowmax]])
    ci = (inputs["var_idx"].astype(np.int64) * 2
          + inputs["sign_idx"].astype(np.int64))              # [B, C, L]
    bx = g[ci].sum(axis=2).max()
    alpha = _pow2(120.0, bx)

    # vq scale from a Cauchy-Schwarz bound: |vq| <= sqrt(D)*max_col ||WvF||
    s_vq = _pow2(120.0, 16.0 * np.sqrt((WvF ** 2).sum(axis=0)).max())
    kz = s_vq / ZV                                            # folded into owt

    m = alpha * s_t * s_kq
    scl = np.zeros((128, 6), np.float32)
    scl[:, 0] = alpha
    scl[:, 1] = s_kq * s_kq / D           # ssq_meas = (alpha*s_t)^2*ssq
    scl[:, 2] = EPS * m * m
    scl[:, 3] = s_kq * s_vq / s_v
    scl[:, 4] = 1.0 / D
    scl[:, 5] = EPS

    owt = np.ascontiguousarray(
        (out_w.T / kz).reshape(2, 128, D).transpose(1, 0, 2)).astype(np.float16)
    bvF = cn_b @ Wv.T + bv                       # bk dropped (softmax shift)
    pqb = (pq + out_b + bvF @ out_w.T).astype(np.float32)
    png = np.broadcast_to(pn_g, (P, D)).astype(np.float32)
    pnb = np.broadcast_to(pn_b, (P, D)).astype(np.float32)
    zc8 = np.full((128, CB), ZV, E4NP)
    ones8 = np.ones((128, 2, 1), E5NP)

    maskb_full = np.where(np.asarray(inputs["mask"]) > 0, 0.0, -1e9)
    lnse = math.log(S_E)

    in_maps = []
    for core in range(NCORES):
        cnt8 = np.zeros((BPC, NCC, 128, 4, 512), E4NP)
        mkb = np.zeros((BPC, 128, CB), np.float32)
        for bl in range(BPC):
            bg = core * BPC + bl
            flat = ci[bg].reshape(-1)
            rows = np.repeat(np.arange(C, dtype=np.int64), L)
            cc = np.bincount(rows * 640 + flat,
                             minlength=C * 640).reshape(C, 640).astype(np.float64)
            full = cc[:, :512] + np.outer(cc[:, 512], w1f) + np.outer(cc[:, 513], w2f)
            full = full.T.reshape(4, 128, C).transpose(1, 0, 2)   # [128, 4, C]
            cnt8[bl] = _e4(full.reshape(128, 4, NCC, 512).transpose(2, 0, 1, 3))
            mkb[bl] = maskb_full[bg].reshape(CB, 128).T + lnse
        in_maps.append({
            "tbl8": tbl8, "wkv8": wkv8, "owt": owt,
            "pqb": pqb, "png": png, "pnb": pnb, "scl": scl, "zc8": zc8,
            "ones8": ones8, "cnt8": cnt8, "maskb": mkb,
        })
    return in_maps


def kernel(**inputs):
    nc = _get_nc()
    in_maps = host_prepare(inputs)
    res = run_bass_kernel_spmd(nc, in_maps, core_ids=list(range(NCORES)))
    out = np.concatenate([res.results[i]["out"] for i in range(NCORES)], axis=0)
    return np.ascontiguousarray(out.astype(np.float32))


# revision 12
# speedup vs baseline: 1.6190x; 1.0283x over previous
"""Trainium2 Bass kernel for nn_CNFAdapter (fp8 DoubleRow, v3).

Algorithm (identical math to the reference, heavily folded):

  The literal MLP ``h = gelu([ve[v]; se[s]] @ W1.T + b1) @ W2.T + b2`` has
  only 514 distinct inputs (257 vars x 2 signs); it is folded on the host
  into a table ``T[514, 256]``.  Dividing T by L bakes in the clause mean;
  subtracting each row's d-mean makes clause vectors exactly zero-mean,
  removing the LN mean term.  Rows 512/513 are folded EXACTLY into the
  first 512 rows via least squares (T[512] = T[:512]^T w has exact
  solutions since rank(T[:512]) = 256), turning the histogram into a
  512-row matrix => the contraction is 4 clean 128-row chunks.

  Per instance (c = clause, d = hidden, hp = (head,query), he = (head,e)):
     xT[d, c]   = tableT @ counts'      (counts' = folded literal histogram)
     rs[c]      = 1/sqrt(sum_d x^2 / D + eps)      (DVE magic-rsqrt+Newton)
     sv[c, :]   = x @ [Wkq | WvF]       (Wkq folds cn_g, Wk, q, softmax scale)
     expT[c,hp] = exp(rs*sv_kq + maskbias)   (bk dropped: softmax shift inv.)
     vq[c, he]  = rs2 * sv_v
     bigctx[hp, he+1] = expT.T @ [vq | zconst]   (Z rides in the extra column)
     ctx[p, he] = diag head blocks / Z
     out        = LN(pqb + ctx @ out_w.T) * pn_g + pn_b

  All big matmuls are fp8e4 MatmulPerfMode.DoubleRow (measured 2x: a K=256
  DR matmul costs one K=128 fp16 matmul).  The v-projection weights use an
  error-compensated hi+lo fp8 pair (plain fp8 there is ~2e-2 rel err).
  De-scales fold into existing operands (exp bias takes ln(s_e), the Z
  column carries the vq scale, a residual power of two rides on out_w).
  The scalar engine runs only table-free Copy plus Exp (no activation-table
  thrash); sqrt/rsqrt run on the vector engine via bitcast magic-rsqrt.

  Sharding: data-parallel over B=32 instances, 4 per NeuronCore; params
  replicated.
"""

import math
from contextlib import ExitStack

import numpy as np
import ml_dtypes

import concourse.bass as bass
import concourse.mybir as mybir
import concourse.tile as tile
from concourse import bacc
from concourse.bass_utils import run_bass_kernel_spmd

# ---------------- problem constants (hardcoded) ----------------
D = 256
H = 8
P = 32
V = 257
EPS = 1e-5
B, C, L = 32, 2048, 8
NCORES = 8
BPC = B // NCORES      # 4 instances per core
CB = C // 128          # 16 chunks of 128 clauses
NCC = 4                # clause column chunks of 512 for xT
hd = D // H

fp8 = mybir.dt.float8e4
fp8e5 = mybir.dt.float8e5
fp16 = mybir.dt.float16
fp32 = mybir.dt.float32
i32 = mybir.dt.int32
AF = mybir.ActivationFunctionType
ALU = mybir.AluOpType
AX = mybir.AxisListType
DR = mybir.MatmulPerfMode.DoubleRow

E4NP = ml_dtypes.float8_e4m3
E5NP = ml_dtypes.float8_e5m2
S_E = 32.0             # exp output scale (via +ln(S_E) on the bias)
ZV = 64.0              # Z-column constant (s_vq * 2^-kz)
SQ8 = 2.0 ** -7        # x28 = (x8*SQ8)*x8, keeps squares in fp8 range
MAGIC = 0x5F3759DF


def _rsqrt(nc, pool, yv, n_part, nfree, tag):
    """rs = 1/sqrt(yv) on the vector engine: magic bitcast + 1 Newton."""
    r0i = pool.tile([n_part, nfree], i32, tag=tag + "i")
    nc.vector.tensor_scalar(r0i[:], yv.bitcast(i32), 1, None,
                            op0=ALU.arith_shift_right)
    r0j = pool.tile([n_part, nfree], i32, tag=tag + "j")
    nc.vector.tensor_scalar(r0j[:], r0i[:], -1, MAGIC,
                            op0=ALU.mult, op1=ALU.add)
    r0 = r0j[:].bitcast(fp32)
    t1 = pool.tile([n_part, nfree], fp32, tag=tag + "a")
    nc.vector.tensor_mul(t1[:], r0, r0)
    t2 = pool.tile([n_part, nfree], fp32, tag=tag + "b")
    nc.vector.tensor_mul(t2[:], t1[:], yv)
    t3 = pool.tile([n_part, nfree], fp32, tag=tag + "c")
    nc.vector.tensor_scalar(t3[:], t2[:], -0.5, 1.5, op0=ALU.mult, op1=ALU.add)
    rs = pool.tile([n_part, nfree], fp32, tag=tag + "r")
    nc.vector.tensor_mul(rs[:], r0, t3[:])
    return rs


def _emit(nc, tc, ctx, dr, out_dram):
    pc = ctx.enter_context(tc.tile_pool(name="consts", bufs=1))
    pcnt = ctx.enter_context(tc.tile_pool(name="cnt", bufs=2))
    px = ctx.enter_context(tc.tile_pool(name="x", bufs=2))
    px2 = ctx.enter_context(tc.tile_pool(name="x2", bufs=2))
    pexp = ctx.enter_context(tc.tile_pool(name="expv", bufs=2))
    pst = ctx.enter_context(tc.tile_pool(name="stats", bufs=2))
    psm = ctx.enter_context(tc.tile_pool(name="small", bufs=2))
    ps_mm = ctx.enter_context(tc.tile_pool(name="ps_mm", bufs=4, space="PSUM"))
    ps_bc = ctx.enter_context(tc.tile_pool(name="ps_bc", bufs=1, space="PSUM"))
    ps_stail = ctx.enter_context(tc.tile_pool(name="ps_stail", bufs=1, space="PSUM"))

    # ---- constants: the xT-critical ones first, then inst-0 counts ----
    tbl8 = pc.tile([128, 4, D], fp8, tag="tbl8")
    nc.sync.dma_start(out=tbl8[:], in_=dr["tbl8"][:])
    scl = pc.tile([128, 6], fp32, tag="scl")  # xsc, stu, stw, k2, 1/D, eps
    nc.sync.dma_start(out=scl[:], in_=dr["scl"][:])
    cnt0 = pcnt.tile([128, 4, C], fp8, tag="cnt")
    for cc in range(NCC):
        eng = nc.sync if cc % 2 == 0 else nc.scalar
        eng.dma_start(out=cnt0[:, :, cc * 512:(cc + 1) * 512],
                      in_=dr["cnt8"][0, cc])
    ones8 = pc.tile([128, 2, 1], fp8e5, tag="ones8")
    nc.scalar.dma_start(out=ones8[:], in_=dr["ones8"][:])
    wkv8 = pc.tile([128, 2, 3, D], fp8, tag="wkv8")
    nc.scalar.dma_start(out=wkv8[:], in_=dr["wkv8"][:])
    owt = pc.tile([128, 2, D], fp16, tag="owt")
    nc.scalar.dma_start(out=owt[:], in_=dr["owt"][:])
    pqb = pc.tile([P, D], fp16, tag="pqb")
    nc.scalar.dma_start(out=pqb[:], in_=dr["pqb"][:])
    ident = pc.tile([P, P], fp16, tag="ident")
    nc.scalar.dma_start(out=ident[:], in_=dr["ident"][:])
    png = pc.tile([P, D], fp32, tag="png")
    nc.scalar.dma_start(out=png[:], in_=dr["png"][:])
    pnb = pc.tile([P, D], fp32, tag="pnb")
    nc.scalar.dma_start(out=pnb[:], in_=dr["pnb"][:])
    zc8 = pc.tile([128, CB], fp8, tag="zc8")
    nc.scalar.dma_start(out=zc8[:], in_=dr["zc8"][:])

    for b in range(BPC):
        # ---- load per-instance data (cnt chunked so compute starts early) ----
        if b == 0:
            cnt = cnt0
        else:
            cnt = pcnt.tile([128, 4, C], fp8, tag="cnt")
            for cc in range(NCC):
                eng = nc.sync if cc % 2 == 0 else nc.scalar
                eng.dma_start(out=cnt[:, :, cc * 512:(cc + 1) * 512],
                              in_=dr["cnt8"][b, cc])
        mb_t = psm.tile([128, CB], fp32, tag="mb")
        nc.sync.dma_start(out=mb_t[:], in_=dr["maskb"][b])

        # ---- xT[d%128, d//128, c] = tableT @ counts (fp8 DoubleRow) ----
        x8 = px.tile([128, 2, C], fp8, tag="x8")
        x2 = px2.tile([128, 2, C], fp8e5, tag="x2")
        pst_t = ps_stail.tile([128, CB], fp32, tag="st")
        for cc in range(NCC):
            csl = slice(cc * 512, (cc + 1) * 512)
            for dh in range(2):
                dsl = slice(dh * 128, (dh + 1) * 128)
                pxm = ps_mm.tile([128, 512], fp32, tag="mm")
                for pr in range(2):
                    nc.tensor.matmul(pxm[:],
                                     lhsT=tbl8[:, 2 * pr:2 * pr + 2, dsl],
                                     rhs=cnt[:, 2 * pr:2 * pr + 2, csl],
                                     start=(pr == 0), stop=(pr == 1),
                                     perf_mode=DR)
                if dh == 0:
                    nc.scalar.activation(x8[:, dh, csl], pxm[:], AF.Copy,
                                         scale=scl[:, 0:1])
                else:
                    nc.vector.tensor_scalar_mul(x8[:, dh, csl], pxm[:],
                                                scl[:, 0:1])
                nc.gpsimd.tensor_mul(x2[:, dh, csl], x8[:, dh, csl],
                                     x8[:, dh, csl])
            for cb in range(4 * cc, 4 * cc + 4):
                nc.tensor.matmul(pst_t[:, cb:cb + 1],
                                 lhsT=x2[:, :, cb * 128:(cb + 1) * 128],
                                 rhs=ones8[:], start=True, stop=True,
                                 perf_mode=DR)
        yv = pst.tile([128, CB], fp32, tag="yv")
        nc.vector.tensor_scalar(yv[:], pst_t[:], scl[:, 1:2], scl[:, 2:3],
                                op0=ALU.mult, op1=ALU.add)
        rs1 = _rsqrt(nc, pst, yv[:], 128, CB, "rq")
        rs2 = pst.tile([128, CB], fp32, tag="rs2")
        nc.vector.tensor_scalar_mul(rs2[:], rs1[:], scl[:, 3:4])

        # ---- s1v per clause chunk; exp / v-scale epilogues; bigctx ----
        expT = pexp.tile([128, CB, 256], fp8, tag="expT")
        vqa = pexp.tile([128, CB, 257], fp8, tag="vqa")
        nc.sync.dma_start(out=vqa[:, :, 256:257], in_=dr["zc8"][:])
        pbc0 = ps_bc.tile([128, 257], fp32, tag="bc0")
        pbc1 = ps_bc.tile([128, 257], fp32, tag="bc1")
        pbc = [pbc0, pbc1]
        for cb in range(CB):
            sv = ps_mm.tile([128, 512], fp32, tag="mm")
            lhsT = x8[:, :, cb * 128:(cb + 1) * 128]
            nc.tensor.matmul(sv[:], lhsT=lhsT, rhs=wkv8[:, :, 0:2, :],
                             start=True, stop=True, perf_mode=DR)
            nc.tensor.matmul(sv[:, 256:512], lhsT=lhsT, rhs=wkv8[:, :, 2, :],
                             start=False, stop=True, perf_mode=DR,
                             skip_group_check=True)
            nc.scalar.activation(expT[:, cb, :], sv[:, 0:256], AF.Exp,
                                 bias=mb_t[:, cb:cb + 1], scale=rs1[:, cb:cb + 1])
            nc.vector.tensor_scalar_mul(vqa[:, cb, 0:256], sv[:, 256:512],
                                        rs2[:, cb:cb + 1])
            if cb % 2 == 1:
                i = cb // 2
                for hpc in range(2):
                    nc.tensor.matmul(pbc[hpc][:],
                                     lhsT=expT[:, cb - 1:cb + 1,
                                               hpc * 128:(hpc + 1) * 128],
                                     rhs=vqa[:, cb - 1:cb + 1, :],
                                     start=(i == 0), stop=(i == CB // 2 - 1),
                                     perf_mode=DR)

        # ---- ctx extraction: scale diag head blocks by 1/Z, transpose ----
        zr = pst.tile([128, 2], fp32, tag="zr")
        for hpc in range(2):
            nc.vector.reciprocal(zr[:, hpc:hpc + 1], pbc[hpc][:, 256:257])
        sc16 = psm.tile([128, 2, P], fp16, tag="sc16")
        ctxT = psm.tile([128, 2, P], fp16, tag="ctxT")
        for h in range(H):
            ch, o = divmod(h, 4)
            osl = slice(o * 32, o * 32 + 32)
            nc.scalar.activation(sc16[osl, ch, :],
                                 pbc[ch][osl, h * 32:h * 32 + 32], AF.Copy,
                                 scale=zr[osl, ch:ch + 1])
        for o in range(4):
            osl = slice(o * 32, o * 32 + 32)
            nc.vector.transpose(ctxT[osl, :, :].rearrange("p a b -> p (a b)"),
                                sc16[osl, :, :].rearrange("p a b -> p (a b)"))

        # ---- refined = ctx @ out_w.T (fp16) ; + pqb ; LayerNorm ----
        prf_t = ps_stail.tile([P, D], fp32, tag="tail")
        for ch in range(2):
            nc.tensor.matmul(prf_t[:], lhsT=ctxT[:, ch, :], rhs=owt[:, ch, :],
                             start=(ch == 0), stop=False)
        nc.tensor.matmul(prf_t[:], lhsT=ident[:], rhs=pqb[:],
                         start=False, stop=True)
        ssum = pst.tile([P, 1], fp32, tag="ssum")
        nc.vector.tensor_reduce(ssum[:], prf_t[:], axis=AX.X, op=ALU.add)
        nm = pst.tile([P, 1], fp32, tag="nm")
        nc.vector.tensor_scalar_mul(nm[:], ssum[:], -1.0 / D)
        cen = psm.tile([P, D], fp32, tag="cen")
        nc.vector.tensor_scalar_add(cen[:], prf_t[:], nm[:, 0:1])
        sq2 = psm.tile([P, D], fp32, tag="sq2")
        nc.gpsimd.tensor_mul(sq2[:], cen[:], cen[:])
        vs = pst.tile([P, 1], fp32, tag="vs")
        nc.vector.tensor_reduce(vs[:], sq2[:], axis=AX.X, op=ALU.add)
        yt = pst.tile([P, 1], fp32, tag="yt")
        nc.vector.tensor_scalar(yt[:], vs[:], 1.0 / D, EPS,
                                op0=ALU.mult, op1=ALU.add)
        rstd = _rsqrt(nc, pst, yt[:], P, 1, "rt")
        t1 = psm.tile([P, D], fp32, tag="t1")
        nc.vector.scalar_tensor_tensor(t1[:], in0=cen[:], scalar=rstd[:, 0:1],
                                       in1=png[:], op0=ALU.mult, op1=ALU.mult)
        outt = psm.tile([P, D], fp32, tag="outt")
        nc.gpsimd.tensor_add(outt[:], t1[:], pnb[:])
        nc.sync.dma_start(out=out_dram[b], in_=outt[:])


def _build_nc():
    nc = bacc.Bacc("TRN2", target_bir_lowering=False, debug=False,
                   num_devices=NCORES)
    dr = {}

    def din(name, shape, dt):
        dr[name] = nc.dram_tensor(name, shape, dt, kind="ExternalInput").ap()

    din("tbl8", [128, 4, D], fp8)
    din("wkv8", [128, 2, 3, D], fp8)
    din("owt", [128, 2, D], fp16)
    din("pqb", [P, D], fp16)
    din("ident", [P, P], fp16)
    din("png", [P, D], fp32)
    din("pnb", [P, D], fp32)
    din("scl", [128, 6], fp32)
    din("zc8", [128, CB], fp8)
    din("ones8", [128, 2, 1], fp8e5)
    din("cnt8", [BPC, NCC, 128, 4, 512], fp8)
    din("maskb", [BPC, 128, CB], fp32)
    out_dram = nc.dram_tensor("out", [BPC, P, D], fp32, kind="ExternalOutput").ap()

    with tile.TileContext(nc) as tc, ExitStack() as ctx:
        _emit(nc, tc, ctx, dr, out_dram)
    nc.compile()
    return nc


_NC_CACHE = None


def _get_nc():
    global _NC_CACHE
    if _NC_CACHE is None:
        _NC_CACHE = _build_nc()
    return _NC_CACHE


def _erf(x):
    try:
        from scipy.special import erf
        return erf(x)
    except Exception:
        from math import erf as _e
        return np.vectorize(_e)(x)


def _pow2(target, absmax):
    if absmax <= 0:
        return 1.0
    return 2.0 ** math.floor(math.log2(target / absmax))


def _e4(x):
    return np.asarray(x, np.float32).astype(E4NP)


def host_prepare(inputs):
    """Fold weights, quantize to fp8, build per-core input maps."""
    ve = inputs["var_embed"].astype(np.float64)
    se = inputs["sign_embed"].astype(np.float64)
    W1 = inputs["W1"].astype(np.float64)
    b1 = inputs["b1"].astype(np.float64)
    W2 = inputs["W2"].astype(np.float64)
    b2 = inputs["b2"].astype(np.float64)
    cn_g = inputs["cn_g"].astype(np.float64)
    cn_b = inputs["cn_b"].astype(np.float64)
    pq = inputs["prefix_queries"].astype(np.float64)
    in_w = inputs["in_proj_w"].astype(np.float64)
    in_b = inputs["in_proj_b"].astype(np.float64)
    out_w = inputs["out_w"].astype(np.float64)
    out_b = inputs["out_b"].astype(np.float64)
    pn_g = inputs["pn_g"].astype(np.float64)
    pn_b = inputs["pn_b"].astype(np.float64)

    # literal table over combined index j = v*2 + s; /L bakes the clause mean,
    # row-centering makes clause vectors exactly zero-mean under LN
    lit = np.concatenate([np.repeat(ve, 2, axis=0), np.tile(se, (V, 1))], axis=1)
    z = lit @ W1.T + b1
    gelu = 0.5 * z * (1.0 + _erf(z / math.sqrt(2.0)))
    table = (gelu @ W2.T + b2) / L
    table = table - table.mean(axis=1, keepdims=True)        # [514, D]

    # fold rows 512/513 exactly into the first 512 (min-norm least squares)
    A = table[:512].T                                        # [256, 512]
    w1f, *_ = np.linalg.lstsq(A, table[512], rcond=None)
    w2f, *_ = np.linalg.lstsq(A, table[513], rcond=None)

    s_t = _pow2(120.0, np.abs(table[:512]).max())
    ts = table[:512] * s_t
    tbl8 = _e4(np.ascontiguousarray(ts.reshape(4, 128, D).transpose(1, 0, 2)))

    Wq, Wk, Wv = np.split(in_w, 3, axis=0)
    bq, bk, bv = np.split(in_b, 3)
    q = pq @ Wq.T + bq                                       # [P, D]
    scale = 1.0 / math.sqrt(hd)
    WkF = cn_g[:, None] * Wk.T
    WvF = cn_g[:, None] * Wv.T
    qh = q.reshape(P, H, hd)
    qbd = np.zeros((D, H * P))
    for h in range(H):
        qbd[h * hd:(h + 1) * hd, h * P:(h + 1) * P] = qh[:, h, :].T * scale
    WKQ = WkF @ qbd                                          # [D, 256]

    s_kq = _pow2(120.0, np.abs(WKQ).max())
    s_v = _pow2(120.0, np.abs(WvF).max())
    wkq8 = _e4(WKQ * s_kq)
    wv_hi = _e4(WvF * s_v)
    wv_lo = _e4(WvF * s_v - wv_hi.astype(np.float64))
    wkv8 = np.zeros((128, 2, 3, D), E4NP)
    for dh in range(2):
        dsl = slice(dh * 128, (dh + 1) * 128)
        wkv8[:, dh, 0, :] = wkq8[dsl]
        wkv8[:, dh, 1, :] = wv_hi[dsl]
        wkv8[:, dh, 2, :] = wv_lo[dsl]

    # x8 scale from a per-clause bound on |x| (g extends rowmax to the fold)
    tq = tbl8.transpose(1, 0, 2).reshape(512, D).astype(np.float64)  # *s_t
    rowmax = np.abs(tq).max(axis=1)                           # [512]
    g = np.concatenate([rowmax,
                        [np.abs(w1f) @ rowmax, np.abs(w2f) @ rowmax]])
    ci = (inputs["var_idx"].astype(np.int64) * 2
          + inputs["sign_idx"].astype(np.int64))              # [B, C, L]
    bx = g[ci].sum(axis=2).max()
    alpha = _pow2(120.0, bx)

    # vq scale from a Cauchy-Schwarz bound: |vq| <= sqrt(D)*max_col ||WvF||
    s_vq = _pow2(120.0, 16.0 * np.sqrt((WvF ** 2).sum(axis=0)).max())
    kz = s_vq / ZV                                            # folded into owt

    m = alpha * s_t * s_kq
    scl = np.zeros((128, 6), np.float32)
    scl[:, 0] = alpha
    scl[:, 1] = s_kq * s_kq / D           # ssq_meas = (alpha*s_t)^2*ssq
    scl[:, 2] = EPS * m * m
    scl[:, 3] = s_kq * s_vq / s_v
    scl[:, 4] = 1.0 / D
    scl[:, 5] = EPS

    owt = np.ascontiguousarray(
        (out_w.T / kz).reshape(2, 128, D).transpose(1, 0, 2)).astype(np.float16)
    bvF = cn_b @ Wv.T + bv                       # bk dropped (softmax shift)
    pqb = (pq + out_b + bvF @ out_w.T).astype(np.float16)
    ident = np.eye(P, dtype=np.float16)
    png = np.broadcast_to(pn_g, (P, D)).astype(np.float32)
    pnb = np.broadcast_to(pn_b, (P, D)).astype(np.float32)
    zc8 = np.full((128, CB), ZV, E4NP)
    ones8 = np.ones((128, 2, 1), E5NP)

    maskb_full = np.where(np.asarray(inputs["mask"]) > 0, 0.0, -1e9)
    lnse = math.log(S_E)

    in_maps = []
    for core in range(NCORES):
        cnt8 = np.zeros((BPC, NCC, 128, 4, 512), E4NP)
        mkb = np.zeros((BPC, 128, CB), np.float32)
        for bl in range(BPC):
            bg = core * BPC + bl
            flat = ci[bg].reshape(-1)
            rows = np.repeat(np.arange(C, dtype=np.int64), L)
            cc = np.bincount(rows * 640 + flat,
                             minlength=C * 640).reshape(C, 640).astype(np.float64)
            full = cc[:, :512] + np.outer(cc[:, 512], w1f) + np.outer(cc[:, 513], w2f)
            full = full.T.reshape(4, 128, C).transpose(1, 0, 2)   # [128, 4, C]
            cnt8[bl] = _e4(full.reshape(128, 4, NCC, 512).transpose(2, 0, 1, 3))
            mkb[bl] = maskb_full[bg].reshape(CB, 128).T + lnse
        in_maps.append({
            "tbl8": tbl8, "wkv8": wkv8, "owt": owt,
            "pqb": pqb, "ident": ident, "png": png, "pnb": pnb, "scl": scl, "zc8": zc8,
            "ones8": ones8, "cnt8": cnt8, "maskb": mkb,
        })
    return in_maps


def kernel(**inputs):
    nc = _get_nc()
    in_maps = host_prepare(inputs)
    res = run_bass_kernel_spmd(nc, in_maps, core_ids=list(range(NCORES)))
    out = np.concatenate([res.results[i]["out"] for i in range(NCORES)], axis=0)
    return np.ascontiguousarray(out.astype(np.float32))
